# revision 1
# baseline (speedup 1.0000x reference)
"""Trainium2 Bass kernel for nn_BSAM_60129542251.

Conv-QKV self-attention block (B=4, C=64, H=W=64):
  Q = conv3x3(A1_B, w1)  -> [b, 32, 4096]
  K = conv3x3(A1_C, w2)  -> [b, 32, 4096]
  V = conv3x3(A1_C, w3)  -> [b, 64, 4096]
  E = softmax(Q^T K) V^T -> [b, 4096, 64];  out = E^T + A1_C

Sharding: 8 cores; core i handles sample b=i//2, row-half i%2 (2048 query
rows). K/V convs are duplicated within a sample pair; Q conv runs on the
core's half only. Attention is fully fused on-chip (no S matrix in HBM).

Attention structure (per 512-row m-tile, key-groups of 4x128 keys):
  ST[4 chunks] = K_k^T Q via 4 row-packed matmuls (contraction is only 32,
  so tile_position packs 4 key-chunks into the 128-row PE array; K lives
  pre-swizzled in 4 partition groups, Q is replicated into all 4 groups by
  the Q-conv using 4x-replicated weights). Two 1024-wide exps on ACT, then
  E'^T += V'_k^T P with V' = [V | ones] so row 64 of E'^T accumulates the
  softmax denominators. Normalize via reciprocal + gpsimd partition
  broadcast, add residual, DMA out.

Precision: matmul inputs in float32r (12-bit mantissa, fp32 range; exact
products, fp32 PSUM accumulation). exp needs no max-subtraction: S range
(|S| < ~50) is exact in fp32/ACT, and denominators stay in fp32 range.

Conv = 9 shifted matmuls over a flat zero-padded input; the row-wrap reads
at w=0/w=63 are cancelled by compact edge-correction matmuls (contiguous
host-gathered edge columns; fp32r matmuls with tiny N are broken in HW, and
a PSUM accumulation group's first matmul clears the whole bank, hence fp32
corrections in a single accumulation group, applied via DVE).
"""

import numpy as np

import concourse.bacc as bacc
import concourse.mybir as mybir
import concourse.tile as tile
from concourse import bass_utils
from concourse.masks import make_identity

F32 = mybir.dt.float32
F32R = mybir.dt.float32r
AF = mybir.ActivationFunctionType

B, C, CH, H, W = 4, 64, 32, 64, 64
N = H * W                     # 4096 keys
M = N // 2                    # 2048 query rows per core
NCORES = 8
XC_LEN = 4352                 # padded flat A1_C: 66*64+2 = 4226, padded up
XB_LEN = 2304                 # padded flat A1_B half: 34*64+2 = 2178, padded up
NKC = N // 128                # 32 key chunks
NKG = NKC // 4                # 8 key groups (4 chunks row-packed per group)
MTA = 512                     # attention m-tile

_cache = {}


def _r32r(x):
    """Round fp32 -> float32r (zero low 12 mantissa bits, round to nearest)."""
    x = np.ascontiguousarray(x, np.float32)
    b = x.view(np.uint32).astype(np.uint64)
    out = (((b + np.uint64(1 << 11)) & np.uint64(0xFFFFF000)).astype(np.uint32)).view(np.float32)
    return np.ascontiguousarray(out)


def _build(dbg=False):
    nc = bacc.Bacc("TRN2", target_bir_lowering=False, debug=False)

    xc = nc.dram_tensor("xc", [128, XC_LEN], F32R, kind="ExternalInput")
    xb = nc.dram_tensor("xb", [128, XB_LEN], F32R, kind="ExternalInput")
    w1t = nc.dram_tensor("w1t", [128, 6 * 128], F32R, kind="ExternalInput")
    w23t = nc.dram_tensor("w23t", [128, 6 * 128], F32R, kind="ExternalInput")
    wc1 = nc.dram_tensor("wc1", [C, 6 * 128], F32R, kind="ExternalInput")
    wc23 = nc.dram_tensor("wc23", [C, 6 * 128], F32R, kind="ExternalInput")
    ecl = nc.dram_tensor("ecl", [C, 66], F32R, kind="ExternalInput")
    ecr = nc.dram_tensor("ecr", [C, 67], F32R, kind="ExternalInput")
    ebl = nc.dram_tensor("ebl", [C, 34], F32R, kind="ExternalInput")
    ebr = nc.dram_tensor("ebr", [C, 35], F32R, kind="ExternalInput")
    b1v = nc.dram_tensor("b1v", [128, 1], F32, kind="ExternalInput")
    b23v = nc.dram_tensor("b23v", [128, 1], F32, kind="ExternalInput")
    resid = nc.dram_tensor("resid", [C, M], F32, kind="ExternalInput")
    out_d = nc.dram_tensor("out", [C, M], F32, kind="ExternalOutput")
    if dbg:
        k_d = nc.dram_tensor("k_dbg", [CH, N], F32, kind="ExternalOutput")
        q_d = nc.dram_tensor("q_dbg", [128, M], F32, kind="ExternalOutput")
        v_d = nc.dram_tensor("v_dbg", [128, NKC * 65], F32, kind="ExternalOutput")

    with tile.TileContext(nc) as tc:
        with (
            tc.tile_pool(name="big", bufs=1) as big,
            tc.tile_pool(name="work", bufs=2) as work,
            tc.tile_pool(name="expool", bufs=4) as expool,
        ):
            xc_sb = big.tile([128, XC_LEN], F32R, tag="xc")
            xb_sb = big.tile([128, XB_LEN], F32R, tag="xb")
            w1_sb = big.tile([128, 6 * 128], F32R, tag="w1")
            w23_sb = big.tile([128, 6 * 128], F32R, tag="w23")
            wc1_sb = big.tile([C, 6 * 128], F32R, tag="wc1")
            wc23_sb = big.tile([C, 6 * 128], F32R, tag="wc23")
            ecl_sb = big.tile([C, 66], F32R, tag="ecl")
            ecr_sb = big.tile([C, 67], F32R, tag="ecr")
            ebl_sb = big.tile([C, 34], F32R, tag="ebl")
            ebr_sb = big.tile([C, 35], F32R, tag="ebr")
            b1_sb = big.tile([128, 1], F32, tag="b1")
            b23_sb = big.tile([128, 1], F32, tag="b23")
            res_sb = big.tile([C, M], F32, tag="res")
            k_sb = big.tile([CH, N], F32R, tag="k")
            k4_sb = big.tile([128, NKG * 128], F32R, tag="k4")
            qt_sb = big.tile([128, M], F32R, tag="qt")
            v_sb = big.tile([128, NKC * 65], F32R, tag="v")
            ident = big.tile([C, C], F32, tag="ident")

            # xb first so conv1 is not gated behind the small loads; the
            # rows 64..127 halves ride the parallel gpsimd (SWDGE) queue
            nc.gpsimd.dma_start(out=xb_sb[C:128, :], in_=xb.ap()[C:128, :])
            nc.sync.dma_start(out=xb_sb[0:C, :], in_=xb.ap()[0:C, :])
            for sb, dr in [
                (w1_sb, w1t), (wc1_sb, wc1), (ebl_sb, ebl), (ebr_sb, ebr),
                (b1_sb, b1v), (b23_sb, b23v),
            ]:
                nc.sync.dma_start(out=sb[:], in_=dr.ap())
            nc.gpsimd.dma_start(out=xc_sb[C:128, :], in_=xc.ap()[C:128, :])
            for sb, dr in [
                (wc23_sb, wc23), (ecl_sb, ecl), (ecr_sb, ecr),
            ]:
                nc.sync.dma_start(out=sb[:], in_=dr.ap())
            nc.sync.dma_start(out=xc_sb[0:C, :], in_=xc.ap()[0:C, :])
            nc.sync.dma_start(out=w23_sb[:], in_=w23t.ap())

            make_identity(nc, ident[:])
            # ones column of V' (col 64 of each chunk): memset fp32, round-copy
            ones_f32 = big.tile([128, NKC], F32, tag="ones")
            nc.vector.memset(ones_f32[:], 1.0)
            # warm the ACT exp table set during the DMA head so the first
            # real exp doesn't pay the ~2.7us table load
            warm = big.tile([128, 1], F32, tag="warm")
            nc.scalar.activation(warm[:], ones_f32[:, 0:1], AF.Exp)
            v3 = v_sb[:].rearrange("p (a b) -> p a b", b=65)
            nc.vector.tensor_copy(v3[:, :, 64], ones_f32[:])

            # ---- convs ----
            with (
                tc.tile_pool(name="cps", bufs=2, space="PSUM") as cps,
                tc.tile_pool(name="vtp", bufs=1, space="PSUM") as vtp,
            ):
                def conv_tile(pk, x_sb, w_sb, wc_sb, el_sb, er_sb, j, co):
                    del wc_sb, el_sb, er_sb
                    for dx in range(3):
                        base = j * 512 + dx
                        nc.tensor.matmul(
                            pk[:], w_sb[:, dx * co:(dx + 1) * co],
                            x_sb[:, base:base + 512],
                            start=(dx == 0), stop=False,
                        )
                    for dx in range(3):
                        base = j * 512 + 2 * 64 + dx
                        nc.tensor.matmul(
                            pk[:], w_sb[0:C, (3 + dx) * co:(4 + dx) * co],
                            x_sb[0:C, base:base + 512],
                            start=False, stop=(dx == 2),
                        )

                # Edge corrections for ALL tiles up front: corrB/corrC hold
                # per-h correction vectors (L in the first half, R in the
                # second); one PSUM accumulation group per bank.
                corrT = vtp.tile([128, 192], F32, tag="corrT")
                corrB = corrT[:, 0:64]
                corrC = corrT[:, 64:192]
                for dy in range(3):
                    nc.tensor.matmul(
                        corrB[:, 0:32],
                        wc1_sb[:, (2 * dy) * 128:(2 * dy + 1) * 128].bitcast(F32),
                        ebl_sb[:, dy:dy + 32].bitcast(F32),
                        start=(dy == 0), stop=False)
                for dy in range(3):
                    nc.tensor.matmul(
                        corrB[:, 32:64],
                        wc1_sb[:, (2 * dy + 1) * 128:(2 * dy + 2) * 128].bitcast(F32),
                        ebr_sb[:, dy + 1:dy + 33].bitcast(F32),
                        start=False, stop=False)
                for dy in range(3):
                    nc.tensor.matmul(
                        corrC[:, 0:64],
                        wc23_sb[:, (2 * dy) * 128:(2 * dy + 1) * 128].bitcast(F32),
                        ecl_sb[:, dy:dy + 64].bitcast(F32),
                        start=False, stop=False)
                for dy in range(3):
                    nc.tensor.matmul(
                        corrC[:, 64:128],
                        wc23_sb[:, (2 * dy + 1) * 128:(2 * dy + 2) * 128].bitcast(F32),
                        ecr_sb[:, dy + 1:dy + 65].bitcast(F32),
                        start=False, stop=(dy == 2))

                # conv1 (Q, 4x-replicated output channels) over this half
                for j in range(4):
                    pq = cps.tile([128, 512], F32, tag="kv")
                    conv_tile(pq, xb_sb, w1_sb, None, None, None, j, 128)
                    qsl = qt_sb[:, j * 512:(j + 1) * 512]
                    nc.scalar.activation(qsl, pq[:], AF.Identity, bias=b1_sb[:])
                    q3 = qsl.rearrange("p (h w) -> p h w", w=64)
                    nc.vector.tensor_add(q3[:, :, 0], q3[:, :, 0], corrB[:, j * 8:(j + 1) * 8])
                    nc.vector.tensor_add(q3[:, :, 63], q3[:, :, 63], corrB[:, 32 + j * 8:32 + (j + 1) * 8])

                # conv2+conv3 fused (K rows 0:32, V rows 64:128) over full n
                for j in range(8):
                    pk = cps.tile([128, 512], F32, tag="kv")
                    conv_tile(pk, xc_sb, w23_sb, None, None, None, j, 128)
                    ksl = k_sb[:, j * 512:(j + 1) * 512]
                    nc.scalar.activation(ksl, pk[0:CH, :], AF.Identity, bias=b23_sb[0:CH, :])
                    k3 = ksl.rearrange("p (h w) -> p h w", w=64)
                    nc.vector.tensor_add(k3[:, :, 0], k3[:, :, 0], corrC[0:CH, j * 8:(j + 1) * 8])
                    nc.vector.tensor_add(k3[:, :, 63], k3[:, :, 63], corrC[0:CH, 64 + j * 8:64 + (j + 1) * 8])
                    vtmp = work.tile([C, 512], F32, tag="vtmp")
                    nc.scalar.activation(vtmp[:], pk[64:128, :], AF.Identity, bias=b23_sb[64:128, :])
                    v3t = vtmp[:].rearrange("p (h w) -> p h w", w=64)
                    nc.vector.tensor_add(v3t[:, :, 0], v3t[:, :, 0], corrC[64:128, j * 8:(j + 1) * 8])
                    nc.vector.tensor_add(v3t[:, :, 63], v3t[:, :, 63], corrC[64:128, 64 + j * 8:64 + (j + 1) * 8])
                    for c4 in range(4):
                        kk = j * 4 + c4
                        tp = cps.tile([128, C], F32, tag="vt")
                        nc.tensor.transpose(tp[:], vtmp[:, c4 * 128:(c4 + 1) * 128], ident[:])
                        nc.vector.tensor_copy(v_sb[:, kk * 65:kk * 65 + 64], tp[:])
                    if j in (3, 7):
                        # swizzle K chunks of tiles j-3..j into the 4 packed
                        # partition groups: k4[32i:, g*128:(g+1)*128] = chunk 4g+i
                        lo = (j - 3) * 512
                        k_re = k_sb[:, (j - 3) * 512:(j + 1) * 512].rearrange(
                            "p (g i c) -> p g i c", i=4, c=128)
                        for i in range(4):
                            nc.sync.dma_start(
                                out=k4_sb[32 * i:32 * (i + 1), lo // 4:lo // 4 + 512],
                                in_=k_re[:, :, i, :],
                            )

            nc.sync.dma_start(out=res_sb[:], in_=resid.ap())

            if dbg:
                nc.sync.dma_start(out=k_d.ap(), in_=k_sb[:].bitcast(F32))
                nc.sync.dma_start(out=q_d.ap(), in_=qt_sb[:].bitcast(F32))
                nc.sync.dma_start(out=v_d.ap(), in_=v_sb[:].bitcast(F32))

            # ---- attention ----
            with (
                tc.tile_pool(name="aps", bufs=3, space="PSUM") as aps,
                tc.tile_pool(name="eps", bufs=1, space="PSUM") as eps,
            ):
                for mt in range(M // MTA):
                    et = eps.tile([65, MTA], F32, tag="et" + str(mt % 2))
                    for kg in range(NKG):
                        # 4 key-chunks row-packed into one PE pass, landing in
                        # two 2-bank st tiles; one exp per st tile.
                        sts = []
                        for h in range(2):
                            st_h = aps.tile([128, 1024], F32, tag="st")
                            sts.append(st_h)
                        for i in range(4):
                            nc.tensor.matmul(
                                sts[i // 2][:, (i % 2) * 512:(i % 2 + 1) * 512],
                                k4_sb[32 * i:32 * (i + 1), kg * 128:(kg + 1) * 128],
                                qt_sb[32 * i:32 * (i + 1), mt * MTA:(mt + 1) * MTA],
                                start=True, stop=True,
                                tile_position=(32 * i, 0),
                            )
                        exs = []
                        for h in range(2):
                            ex = expool.tile([128, 1024], F32R, tag="ex")
                            nc.scalar.activation(ex[:], sts[h][:], AF.Exp)
                            exs.append(ex)
                        for i in range(4):
                            kk = kg * 4 + i
                            nc.tensor.matmul(
                                et[:],
                                v_sb[:, kk * 65:kk * 65 + 65],
                                exs[i // 2][:, (i % 2) * 512:(i % 2 + 1) * 512],
                                start=(kk == 0), stop=(kk == NKC - 1),
                            )
                    recip = work.tile([1, MTA], F32, tag="recip")
                    nc.vector.reciprocal(recip[:], et[64:65, :])
                    bc = work.tile([C, MTA], F32, tag="bc")
                    nc.gpsimd.partition_broadcast(bc[:], recip[:])
                    ot = work.tile([C, MTA], F32, tag="ot")
                    nc.vector.tensor_mul(ot[:], et[0:C, :], bc[:])
                    nc.vector.tensor_add(ot[:], ot[:], res_sb[:, mt * MTA:(mt + 1) * MTA])
                    nc.sync.dma_start(out=out_d.ap()[:, mt * MTA:(mt + 1) * MTA], in_=ot[:])

    nc.compile()
    return nc


def _prep_core_inputs(inputs, core):
    A1_B = np.asarray(inputs["A1_B"], np.float32)
    A1_C = np.asarray(inputs["A1_C"], np.float32)
    w1 = np.asarray(inputs["w1"], np.float32)
    b1 = np.asarray(inputs["b1"], np.float32)
    w2 = np.asarray(inputs["w2"], np.float32)
    b2 = np.asarray(inputs["b2"], np.float32)
    w3 = np.asarray(inputs["w3"], np.float32)
    b3 = np.asarray(inputs["b3"], np.float32)
    b = core // 2
    half = core % 2
    h0 = half * 32

    xc = np.zeros((128, XC_LEN), np.float32)
    flat_c = np.zeros((C, H + 2, W), np.float32)
    flat_c[:, 1:H + 1, :] = A1_C[b]
    xc[0:C, 1:1 + (H + 2) * W] = flat_c.reshape(C, -1)
    xc[C:128, 0:XC_LEN - 64] = xc[0:C, 64:XC_LEN]

    xb = np.zeros((128, XB_LEN), np.float32)
    flat_b = np.zeros((C, 34, W), np.float32)
    glo = h0 - 1
    src_lo = max(glo, 0)
    src_hi = min(h0 + 33, H)
    flat_b[:, src_lo - glo: src_hi - glo, :] = A1_B[b][:, src_lo:src_hi, :]
    xb[0:C, 1:1 + 34 * W] = flat_b.reshape(C, -1)
    xb[C:128, 0:XB_LEN - 64] = xb[0:C, 64:XB_LEN]

    # w1 output channels replicated 4x (partition groups for packed QK).
    # Tap layout: cols t*128.. with t=0..2 the (dy=0,dy=1) stacked pairs
    # (contraction rows 0:64 = dy0, 64:128 = dy1), t=3..5 the dy=2 singles.
    w1t = np.zeros((128, 6 * 128), np.float32)
    w23t = np.zeros((128, 6 * 128), np.float32)
    for dx in range(3):
        for r in range(4):
            sl = slice(dx * 128 + r * CH, dx * 128 + (r + 1) * CH)
            w1t[0:C, sl] = w1[:, :, 0, dx].T
            w1t[C:128, sl] = w1[:, :, 1, dx].T
            sl2 = slice((3 + dx) * 128 + r * CH, (3 + dx) * 128 + (r + 1) * CH)
            w1t[0:C, sl2] = w1[:, :, 2, dx].T
        w23t[0:C, dx * 128: dx * 128 + CH] = w2[:, :, 0, dx].T
        w23t[C:128, dx * 128: dx * 128 + CH] = w2[:, :, 1, dx].T
        w23t[0:C, dx * 128 + 64: dx * 128 + 128] = w3[:, :, 0, dx].T
        w23t[C:128, dx * 128 + 64: dx * 128 + 128] = w3[:, :, 1, dx].T
        w23t[0:C, (3 + dx) * 128: (3 + dx) * 128 + CH] = w2[:, :, 2, dx].T
        w23t[0:C, (3 + dx) * 128 + 64: (3 + dx) * 128 + 128] = w3[:, :, 2, dx].T
    wc1 = np.zeros((C, 6 * 128), np.float32)
    wc23 = np.zeros((C, 6 * 128), np.float32)
    for dy in range(3):
        for side, dx in ((0, 0), (1, 2)):
            i = 2 * dy + side
            for r in range(4):
                wc1[:, i * 128 + r * CH: i * 128 + (r + 1) * CH] = -w1[:, :, dy, dx].T
            wc23[:, i * 128: i * 128 + CH] = -w2[:, :, dy, dx].T
            wc23[:, i * 128 + 64: i * 128 + 128] = -w3[:, :, dy, dx].T

    resid = np.ascontiguousarray(A1_C[b][:, h0:h0 + 32, :].reshape(C, M))
    xcr = _r32r(xc)
    xbr = _r32r(xb)
    return {
        "xc": xcr,
        "xb": xbr,
        "ecl": np.ascontiguousarray(xcr[0:C, (np.arange(66)) * 64]),
        "ecr": np.ascontiguousarray(xcr[0:C, (np.arange(67)) * 64 + 1]),
        "ebl": np.ascontiguousarray(xbr[0:C, (np.arange(34)) * 64]),
        "ebr": np.ascontiguousarray(xbr[0:C, (np.arange(35)) * 64 + 1]),
        "w1t": _r32r(w1t),
        "w23t": _r32r(w23t),
        "wc1": _r32r(wc1),
        "wc23": _r32r(wc23),
        "b1v": np.ascontiguousarray(np.tile(b1, 4).reshape(128, 1)).astype(np.float32),
        "b23v": np.ascontiguousarray(
            np.concatenate([b2, np.zeros(32, np.float32), b3]).reshape(128, 1)),
        "resid": resid,
    }


def _run(inputs, trace=False, dbg=False):
    key = ("nc", dbg)
    if key not in _cache:
        _cache[key] = _build(dbg)
    nc = _cache[key]
    in_maps = [_prep_core_inputs(inputs, i) for i in range(NCORES)]
    res = bass_utils.run_bass_kernel_spmd(
        nc, in_maps, core_ids=list(range(NCORES)), trace=trace)
    out = np.empty((B, C, H, W), np.float32)
    for i in range(NCORES):
        b, half = i // 2, i % 2
        out[b, :, half * 32:half * 32 + 32, :] = res.results[i]["out"].reshape(C, 32, W)
    return out, res


def kernel(**inputs):
    out, _ = _run(inputs, trace=False)
    return out



# revision 14
# speedup vs baseline: 1.1263x; 1.1263x over previous
"""Trainium2 Bass kernel for nn_BSAM_60129542251.

Conv-QKV self-attention block (B=4, C=64, H=W=64):
  Q = conv3x3(A1_B, w1)  -> [b, 32, 4096]
  K = conv3x3(A1_C, w2)  -> [b, 32, 4096]
  V = conv3x3(A1_C, w3)  -> [b, 64, 4096]
  E = softmax(Q^T K) V^T -> [b, 4096, 64];  out = E^T + A1_C

Sharding: 8 cores; core i handles sample b=i//2, row-half i%2 (2048 query
rows). K/V convs are duplicated within a sample pair; Q conv runs on the
core's half only. Attention is fully fused on-chip (no S matrix in HBM).

v3 structure (engine-balanced, conv/attention interleaved, software
pipelined):
  - Conv = 9 shifted matmuls done as 6 (dy-pairs stacked in contraction;
    rows 64:128 of the flat input hold a 64-shifted copy), with compact
    edge-correction matmuls cancelling the w=0/w=63 row-wrap reads.
  - Conv epilogues (PSUM->SBUF + bias) run on Pool (K, Q as fp32 bits into
    f32r tiles) and DVE (V -> bf16), keeping ACT free for exp.
  - V tiles transposed via one batched XBAR DMA transpose per conv tile
    ([64,512] -> [128, 4, 64] 3D out) straight into the [128, 65]-per-chunk
    V' stationary layout (col 64 = ones for the softmax denominator row).
  - Attention per (mt, pair-of-2-chunks): 2 QK matmuls (f32r, k chunk
    [32,128] stationary) into a [128,1024] PSUM tile, one exp -> bf16,
    2 PV matmuls (bf16) accumulating E'^T in PSUM ([65, 512], row 64 =
    denominators). exp ops are split between ACT (exact, scale=1/EXPA)
    and DVE/Pool (2^x int16-bitcast approx; EXPA folded into w1).
  - mt0's attention interleaves with the KV conv, QK one conv tile behind,
    PV two behind (hides the K-epilogue and V-transpose latency). mt1..3
    run after with a QK/PV two-stage pipeline. PSUM: 2 conv + 4 st + 2 et.
  - Normalize tail split in column halves across DVE (recip, mul) and
    Pool (broadcast, add) to shrink the exposed critical path at the end.
  - Inputs ride 2 bundled small-weight DMAs + per-half chunked xb/xc
    (sync HWDGE queue for rows 0:64, gpsimd SWDGE for rows 64:128).
"""

import numpy as np

import concourse.bacc as bacc
import concourse.mybir as mybir
import concourse.tile as tile
from concourse import bass_utils
from concourse.masks import make_identity

F32 = mybir.dt.float32
F32R = mybir.dt.float32r
BF16 = mybir.dt.bfloat16
I16 = mybir.dt.int16
AF = mybir.ActivationFunctionType

B, C, CH, H, W = 4, 64, 32, 64, 64
N = H * W                     # 4096 keys
M = N // 2                    # 2048 query rows per core
NCORES = 8
XC_LEN = 4352                 # padded flat A1_C: 66*64+2 = 4226, padded up
XB_LEN = 2304                 # padded flat A1_B half: 34*64+2 = 2178, padded up
NKC = N // 128                # 32 key chunks
NPAIR = NKC // 2              # 16 chunk pairs (one [128,1024] st tile each)

# bund64 column offsets: wc1 | wc23 | ecl | ecr | ebl | ebr
B64_WC1, B64_WC23, B64_ECL, B64_ECR, B64_EBL, B64_EBR, B64_LEN = (
    0, 192, 960, 1026, 1093, 1127, 1162)
# bund128 column offsets: w1t | w23t | b23 | b1 | bv (V bias at rows 0:64)
B128_W1, B128_W23, B128_B23, B128_B1, B128_BV, B128_LEN = 0, 192, 960, 961, 962, 963

# 2^x trick constants: S_psum = EXPA * s (EXPA folded into w1 on host);
# p = exp(s) ~ bitcast_bf16(int16(S_psum + EXPB)).
EXPA = 128.0 / float(np.log(2.0))          # 184.664965...
EXPB_ROUND = 16256.0 - 5.5053              # round-to-nearest int16 convert
EXPB_TRUNC = EXPB_ROUND + 0.5              # truncate-toward-zero convert


def _exp_engine(mt, p):
    """exp engine for (mt, pair): ACT exact, or DVE 2^x approx."""
    if p % 3 == 2:
        return "dve"
    return "act"


_cache = {}


def _r32r(x):
    """Round fp32 -> float32r (zero low 12 mantissa bits, round to nearest)."""
    x = np.ascontiguousarray(x, np.float32)
    b = x.view(np.uint32).astype(np.uint64)
    out = (((b + np.uint64(1 << 11)) & np.uint64(0xFFFFF000)).astype(np.uint32)).view(np.float32)
    return np.ascontiguousarray(out)


def _build(dbg=False, expb=EXPB_TRUNC):
    nc = bacc.Bacc("TRN2", target_bir_lowering=False, debug=False)

    xc = nc.dram_tensor("xc", [128, XC_LEN], F32R, kind="ExternalInput")
    xb = nc.dram_tensor("xb", [128, XB_LEN], F32R, kind="ExternalInput")
    bund64 = nc.dram_tensor("bund64", [C, B64_LEN], F32R, kind="ExternalInput")
    bund128 = nc.dram_tensor("bund128", [128, B128_LEN], F32R, kind="ExternalInput")
    resid = nc.dram_tensor("resid", [C, M], F32, kind="ExternalInput")
    out_d = nc.dram_tensor("out", [C, M], F32, kind="ExternalOutput")
    if dbg:
        k_d = nc.dram_tensor("k_dbg", [CH, N], F32, kind="ExternalOutput")
        q_d = nc.dram_tensor("q_dbg", [CH, M], F32, kind="ExternalOutput")
        v_d = nc.dram_tensor("v_dbg", [128, NKC * 65], F32, kind="ExternalOutput")

    with tile.TileContext(nc) as tc:
        with (
            tc.tile_pool(name="big", bufs=1) as big,
            tc.tile_pool(name="work", bufs=2) as work,
            tc.tile_pool(name="expool", bufs=6) as expool,
        ):
            xc_sb = big.tile([128, XC_LEN], F32R, tag="xc")
            xb_sb = big.tile([128, XB_LEN], F32R, tag="xb")
            b64_sb = big.tile([C, B64_LEN], F32R, tag="b64")
            b128_sb = big.tile([128, B128_LEN], F32R, tag="b128")
            res_sb = big.tile([C, M], F32, tag="res")
            corrS = big.tile([C, 320], F32, tag="corrS")
            k_sb = big.tile([CH, N], F32R, tag="k")
            qt_sb = big.tile([CH, M], F32R, tag="qt")
            v_sb = big.tile([128, NKC * 65], BF16, tag="v")

            wc1_sb = b64_sb[:, B64_WC1:B64_WC1 + 6 * CH]
            wc23_sb = b64_sb[:, B64_WC23:B64_WC23 + 6 * 128]
            ecl_sb = b64_sb[:, B64_ECL:B64_ECL + 66]
            ecr_sb = b64_sb[:, B64_ECR:B64_ECR + 67]
            ebl_sb = b64_sb[:, B64_EBL:B64_EBL + 34]
            ebr_sb = b64_sb[:, B64_EBR:B64_EBR + 35]
            w1_sb = b128_sb[:, B128_W1:B128_W1 + 6 * CH]
            w23_sb = b128_sb[:, B128_W23:B128_W23 + 6 * 128]
            b23_sb = b128_sb[:, B128_B23:B128_B23 + 1].bitcast(F32)
            b1_sb = b128_sb[0:CH, B128_B1:B128_B1 + 1].bitcast(F32)
            bv_sb = b128_sb[0:C, B128_BV:B128_BV + 1].bitcast(F32)

            # ---- input DMA head ----
            # sync/HWDGE: bundles + rows 0:64 of xb/xc (chunked);
            # gpsimd/SWDGE (Pool desc-gen): rows 64:128 + resid.
            nc.sync.dma_start(out=b64_sb[:], in_=bund64.ap())
            nc.sync.dma_start(out=b128_sb[:], in_=bund128.ap())
            XBC = [0, 704, 1408, XB_LEN]
            for lo, hi in zip(XBC[:-1], XBC[1:]):
                nc.gpsimd.dma_start(out=xb_sb[C:128, lo:hi], in_=xb.ap()[C:128, lo:hi])
                nc.sync.dma_start(out=xb_sb[0:C, lo:hi], in_=xb.ap()[0:C, lo:hi])
            XCC = [0, 1152, 2304, 3456, XC_LEN]
            for lo, hi in zip(XCC[:-1], XCC[1:]):
                nc.gpsimd.dma_start(out=xc_sb[C:128, lo:hi], in_=xc.ap()[C:128, lo:hi])
                nc.sync.dma_start(out=xc_sb[0:C, lo:hi], in_=xc.ap()[0:C, lo:hi])

            # ones columns of V' (col 64 of each chunk) + exp table warm
            ones_bf = big.tile([128, NKC], BF16, tag="ones")
            nc.vector.memset(ones_bf[:], 1.0)
            v3 = v_sb[:].rearrange("p (a b) -> p a b", b=65)
            nc.vector.tensor_copy(v3[:, :, 64], ones_bf[:])
            warm = big.tile([128, 1], F32, tag="warm")
            nc.scalar.activation(warm[:], b23_sb[:], AF.Exp)

            # ---- edge-correction matmuls (own PSUM bank, freed early) ----
            # layout (SBUF partition-aligned with consumers):
            #   Q-L [0:32, 0:32]   Q-R [0:32, 32:64]
            #   K-L [0:32, 64:128] K-R [0:32, 128:192]
            #   V-L [0:64, 192:256] V-R [0:64, 256:320]
            with tc.tile_pool(name="vtp", bufs=1, space="PSUM") as vtp:
                corrT = vtp.tile([C, 320], F32, tag="corrT")
                mms = []
                for dy in range(3):
                    # V first: the group's start zero-region must span the
                    # full partition range (0:64) of the tile.
                    mms.append((corrT[0:C, 192:256],
                                wc23_sb[:, (2 * dy) * 128 + 64:(2 * dy) * 128 + 128],
                                ecl_sb[:, dy:dy + 64]))
                    mms.append((corrT[0:C, 256:320],
                                wc23_sb[:, (2 * dy + 1) * 128 + 64:(2 * dy + 1) * 128 + 128],
                                ecr_sb[:, dy + 1:dy + 65]))
                    mms.append((corrT[0:CH, 0:32],
                                wc1_sb[:, (2 * dy) * CH:(2 * dy + 1) * CH],
                                ebl_sb[:, dy:dy + 32]))
                    mms.append((corrT[0:CH, 32:64],
                                wc1_sb[:, (2 * dy + 1) * CH:(2 * dy + 2) * CH],
                                ebr_sb[:, dy + 1:dy + 33]))
                    mms.append((corrT[0:CH, 64:128],
                                wc23_sb[:, (2 * dy) * 128:(2 * dy) * 128 + CH],
                                ecl_sb[:, dy:dy + 64]))
                    mms.append((corrT[0:CH, 128:192],
                                wc23_sb[:, (2 * dy + 1) * 128:(2 * dy + 1) * 128 + CH],
                                ecr_sb[:, dy + 1:dy + 65]))
                # first and last matmul must span the full 0:64 partition
                # range: start's zero region and stop's group close both
                # cover only the instruction's own partitions.
                mms = [mms[0]] + mms[2:] + [mms[1]]
                for i_mm, (o, l, r) in enumerate(mms):
                    nc.tensor.matmul(o, l.bitcast(F32), r.bitcast(F32),
                                     start=(i_mm == 0), stop=(i_mm == len(mms) - 1))
                nc.vector.tensor_copy(corrS[0:CH, 0:192], corrT[0:CH, 0:192])
                nc.vector.tensor_copy(corrS[0:C, 192:320], corrT[0:C, 192:320])
            corrQ = corrS[0:CH, 0:64]

            def conv_tile(pk, x_sb, w_sb, j, co):
                for dx in range(3):
                    base = j * 512 + dx
                    nc.tensor.matmul(
                        pk[:], w_sb[:, dx * co:(dx + 1) * co],
                        x_sb[:, base:base + 512],
                        start=(dx == 0), stop=False,
                    )
                for dx in range(3):
                    base = j * 512 + 2 * 64 + dx
                    nc.tensor.matmul(
                        pk[:], w_sb[0:C, (3 + dx) * co:(4 + dx) * co],
                        x_sb[0:C, base:base + 512],
                        start=False, stop=(dx == 2),
                    )

            # ---- conv + attention ----
            identb = big.tile([C, C], BF16, tag="identb")
            make_identity(nc, identb[:])

            with tc.tile_pool(name="eps", bufs=2, space="PSUM") as eps:
                ets = {}

                def attn_qk(mt, p, pool):
                    st = pool.tile([128, 1024], F32, tag="st")
                    for i in range(2):
                        kk = 2 * p + i
                        nc.tensor.matmul(
                            st[:, i * 512:(i + 1) * 512],
                            k_sb[:, kk * 128:(kk + 1) * 128],
                            qt_sb[:, mt * 512:(mt + 1) * 512],
                            start=True, stop=True,
                        )
                    ex = expool.tile([128, 1024], BF16, tag="ex")
                    if _exp_engine(mt, p) == "act":
                        nc.scalar.activation(ex[:], st[:], AF.Exp, scale=1.0 / EXPA)
                    else:
                        nc.vector.tensor_scalar_add(ex[:].bitcast(I16), st[:], expb)
                    return ex

                def attn_pv(mt, p, ex):
                    for i in range(2):
                        kk = 2 * p + i
                        nc.tensor.matmul(
                            ets[mt][:],
                            v_sb[:, kk * 65:kk * 65 + 65],
                            ex[:, i * 512:(i + 1) * 512],
                            start=(kk == 0), stop=(kk == NKC - 1),
                        )

                def attn_tail(mt):
                    et = ets[mt]
                    for h in range(2):
                        sl = slice(h * 256, (h + 1) * 256)
                        recip = work.tile([1, 256], F32, tag="recip")
                        nc.vector.reciprocal(recip[:], et[C:C + 1, sl])
                        bc = work.tile([C, 256], F32, tag="bc")
                        nc.gpsimd.partition_broadcast(bc[:], recip[:])
                        ot = work.tile([C, 256], F32, tag="ot")
                        nc.vector.tensor_mul(ot[:], et[0:C, sl], bc[:])
                        nc.gpsimd.tensor_add(ot[:], ot[:], res_sb[:, mt * 512 + h * 256:mt * 512 + (h + 1) * 256])
                        nc.sync.dma_start(out=out_d.ap()[:, mt * 512 + h * 256:mt * 512 + (h + 1) * 256], in_=ot[:])

                # ---- conv phase (attention PSUM not yet open) ----
                with (
                    tc.tile_pool(name="cps", bufs=2, space="PSUM") as cps,
                    tc.tile_pool(name="vtp2", bufs=2, space="PSUM") as vtp2,
                ):
                    def q_conv_tile(j):
                        pq0 = cps.tile([128, 512], F32, tag="cv")
                        pq = pq0[0:CH, :]
                        conv_tile(pq, xb_sb, w1_sb, j, CH)
                        qsl = qt_sb[:, j * 512:(j + 1) * 512]
                        nc.vector.tensor_scalar_add(qsl, pq[:], b1_sb)
                        q3 = qsl.rearrange("p (h w) -> p h w", w=64)
                        nc.vector.tensor_add(q3[:, :, 0], q3[:, :, 0], corrQ[:, j * 8:(j + 1) * 8])
                        nc.vector.tensor_add(q3[:, :, 63], q3[:, :, 63], corrQ[:, 32 + j * 8:32 + (j + 1) * 8])

                    def kv_conv_tile(j):
                        pk = cps.tile([128, 512], F32, tag="cv")
                        conv_tile(pk, xc_sb, w23_sb, j, 128)
                        ksl = k_sb[:, j * 512:(j + 1) * 512]
                        nc.vector.tensor_scalar_add(ksl, pk[0:CH, :], b23_sb[0:CH, :])
                        k3 = ksl.rearrange("p (h w) -> p h w", w=64)
                        nc.gpsimd.tensor_add(k3[:, :, 0], k3[:, :, 0], corrS[0:CH, 64 + j * 8:64 + (j + 1) * 8])
                        nc.gpsimd.tensor_add(k3[:, :, 63], k3[:, :, 63], corrS[0:CH, 128 + j * 8:128 + (j + 1) * 8])
                        vtmp = work.tile([C, 512], BF16, tag="vtmp")
                        nc.scalar.activation(vtmp[:], pk[64:128, :], AF.Identity, bias=bv_sb)
                        v3t = vtmp[:].rearrange("p (h w) -> p h w", w=64)
                        nc.vector.tensor_add(v3t[:, :, 0], v3t[:, :, 0], corrS[0:C, 192 + j * 8:192 + (j + 1) * 8])
                        nc.vector.tensor_add(v3t[:, :, 63], v3t[:, :, 63], corrS[0:C, 256 + j * 8:256 + (j + 1) * 8])
                        vt = vtp2.tile([128, 256], BF16, tag="vt")
                        for c4 in range(4):
                            nc.tensor.transpose(
                                vt[:, c4 * 64:(c4 + 1) * 64],
                                vtmp[:, c4 * 128:(c4 + 1) * 128], identb[:])
                        vslc = v_sb[:, j * 260:(j + 1) * 260]
                        vdst = vslc.rearrange("p (c f) -> p c f", f=65)[:, :, 0:64]
                        nc.vector.tensor_copy(vdst, vt[:].rearrange("p (c f) -> p c f", f=64))

                    for j in range(4):
                        q_conv_tile(j)
                    for j in range(8):
                        kv_conv_tile(j)
                    nc.gpsimd.dma_start(out=res_sb[:], in_=resid.ap())

                if dbg:
                    nc.sync.dma_start(out=k_d.ap(), in_=k_sb[:].bitcast(F32))
                    nc.sync.dma_start(out=q_d.ap(), in_=qt_sb[:].bitcast(F32))
                    vdbg = big.tile([128, NKC * 65], F32, tag="vdbg")
                    nc.vector.tensor_copy(vdbg[:], v_sb[:])
                    nc.sync.dma_start(out=v_d.ap(), in_=vdbg[:])

                # ---- attention phase: all 4 mts, three-stage pipeline ----
                with tc.tile_pool(name="aps2", bufs=3, space="PSUM") as aps2:
                    for mt in range(4):
                        etm = eps.tile([C + 1, 512], F32, tag="et")
                        ets[mt] = etm
                        pend = []
                        for p in range(NPAIR):
                            pend.append((p, attn_qk(mt, p, aps2)))
                            if p == 1 and mt > 0:
                                attn_tail(mt - 1)
                            if len(pend) > 2:
                                pp, pex = pend.pop(0)
                                attn_pv(mt, pp, pex)
                        for pp, pex in pend:
                            attn_pv(mt, pp, pex)
                    attn_tail(3)

    nc.compile()
    return nc


def _prep_core_inputs(inputs, core):
    A1_B = np.asarray(inputs["A1_B"], np.float32)
    A1_C = np.asarray(inputs["A1_C"], np.float32)
    w1 = np.asarray(inputs["w1"], np.float32) * EXPA
    b1 = np.asarray(inputs["b1"], np.float32) * EXPA
    w2 = np.asarray(inputs["w2"], np.float32)
    b2 = np.asarray(inputs["b2"], np.float32)
    w3 = np.asarray(inputs["w3"], np.float32)
    b3 = np.asarray(inputs["b3"], np.float32)
    b = core // 2
    half = core % 2
    h0 = half * 32

    xc = np.zeros((128, XC_LEN), np.float32)
    flat_c = np.zeros((C, H + 2, W), np.float32)
    flat_c[:, 1:H + 1, :] = A1_C[b]
    xc[0:C, 1:1 + (H + 2) * W] = flat_c.reshape(C, -1)
    xc[C:128, 0:XC_LEN - 64] = xc[0:C, 64:XC_LEN]

    xb = np.zeros((128, XB_LEN), np.float32)
    flat_b = np.zeros((C, 34, W), np.float32)
    glo = h0 - 1
    src_lo = max(glo, 0)
    src_hi = min(h0 + 33, H)
    flat_b[:, src_lo - glo: src_hi - glo, :] = A1_B[b][:, src_lo:src_hi, :]
    xb[0:C, 1:1 + 34 * W] = flat_b.reshape(C, -1)
    xb[C:128, 0:XB_LEN - 64] = xb[0:C, 64:XB_LEN]

    # Tap layout: cols t*co.. with t=0..2 the (dy=0,dy=1) stacked pairs
    # (contraction rows 0:64 = dy0, 64:128 = dy1), t=3..5 the dy=2 singles.
    w1t = np.zeros((128, 6 * CH), np.float32)
    w23t = np.zeros((128, 6 * 128), np.float32)
    for dx in range(3):
        w1t[0:C, dx * CH:(dx + 1) * CH] = w1[:, :, 0, dx].T
        w1t[C:128, dx * CH:(dx + 1) * CH] = w1[:, :, 1, dx].T
        w1t[0:C, (3 + dx) * CH:(4 + dx) * CH] = w1[:, :, 2, dx].T
        w23t[0:C, dx * 128: dx * 128 + CH] = w2[:, :, 0, dx].T
        w23t[C:128, dx * 128: dx * 128 + CH] = w2[:, :, 1, dx].T
        w23t[0:C, dx * 128 + 64: dx * 128 + 128] = w3[:, :, 0, dx].T
        w23t[C:128, dx * 128 + 64: dx * 128 + 128] = w3[:, :, 1, dx].T
        w23t[0:C, (3 + dx) * 128: (3 + dx) * 128 + CH] = w2[:, :, 2, dx].T
        w23t[0:C, (3 + dx) * 128 + 64: (3 + dx) * 128 + 128] = w3[:, :, 2, dx].T
    wc1 = np.zeros((C, 6 * CH), np.float32)
    wc23 = np.zeros((C, 6 * 128), np.float32)
    for dy in range(3):
        for side, dx in ((0, 0), (1, 2)):
            i = 2 * dy + side
            wc1[:, i * CH:(i + 1) * CH] = -w1[:, :, dy, dx].T
            wc23[:, i * 128: i * 128 + CH] = -w2[:, :, dy, dx].T
            wc23[:, i * 128 + 64: i * 128 + 128] = -w3[:, :, dy, dx].T

    xcr = _r32r(xc)
    xbr = _r32r(xb)

    bund64 = np.zeros((C, B64_LEN), np.float32)
    bund64[:, B64_WC1:B64_WC1 + 6 * CH] = _r32r(wc1)
    bund64[:, B64_WC23:B64_WC23 + 6 * 128] = _r32r(wc23)
    bund64[:, B64_ECL:B64_ECL + 66] = xcr[0:C, (np.arange(66)) * 64]
    bund64[:, B64_ECR:B64_ECR + 67] = xcr[0:C, (np.arange(67)) * 64 + 1]
    bund64[:, B64_EBL:B64_EBL + 34] = xbr[0:C, (np.arange(34)) * 64]
    bund64[:, B64_EBR:B64_EBR + 35] = xbr[0:C, (np.arange(35)) * 64 + 1]

    bund128 = np.zeros((128, B128_LEN), np.float32)
    bund128[:, B128_W1:B128_W1 + 6 * CH] = _r32r(w1t)
    bund128[:, B128_W23:B128_W23 + 6 * 128] = _r32r(w23t)
    bund128[:, B128_B23] = np.concatenate([b2, np.zeros(32, np.float32), b3])
    bund128[0:CH, B128_B1] = b1
    bund128[0:C, B128_BV] = b3

    resid = np.ascontiguousarray(A1_C[b][:, h0:h0 + 32, :].reshape(C, M))
    return {
        "xc": xcr,
        "xb": xbr,
        "bund64": bund64,
        "bund128": bund128,
        "resid": resid,
    }


def _run(inputs, trace=False, dbg=False):
    key = ("nc", dbg)
    if key not in _cache:
        _cache[key] = _build(dbg)
    nc = _cache[key]
    in_maps = [_prep_core_inputs(inputs, i) for i in range(NCORES)]
    res = bass_utils.run_bass_kernel_spmd(
        nc, in_maps, core_ids=list(range(NCORES)), trace=trace)
    out = np.empty((B, C, H, W), np.float32)
    for i in range(NCORES):
        b, half = i // 2, i % 2
        out[b, :, half * 32:half * 32 + 32, :] = res.results[i]["out"].reshape(C, 32, W)
    return out, res


def kernel(**inputs):
    out, _ = _run(inputs, trace=False)
    return out


# revision 23
# speedup vs baseline: 1.1757x; 1.0438x over previous
"""Trainium2 Bass kernel for nn_BSAM_60129542251.

Conv-QKV self-attention block (B=4, C=64, H=W=64):
  Q = conv3x3(A1_B, w1)  -> [b, 32, 4096]
  K = conv3x3(A1_C, w2)  -> [b, 32, 4096]
  V = conv3x3(A1_C, w3)  -> [b, 64, 4096]
  E = softmax(Q^T K) V^T -> [b, 4096, 64];  out = E^T + A1_C

Sharding: 8 cores; core i handles sample b=i//2, row-half i%2 (2048 query
rows). K/V convs are duplicated within a sample pair; Q conv runs on the
core's half only. Attention is fully fused on-chip (no S matrix in HBM).

v3 structure (engine-balanced, conv/attention interleaved, software
pipelined):
  - Conv = 9 shifted matmuls done as 6 (dy-pairs stacked in contraction;
    rows 64:128 of the flat input hold a 64-shifted copy), with compact
    edge-correction matmuls cancelling the w=0/w=63 row-wrap reads.
  - Conv epilogues (PSUM->SBUF + bias) run on Pool (K, Q as fp32 bits into
    f32r tiles) and DVE (V -> bf16), keeping ACT free for exp.
  - V tiles transposed via one batched XBAR DMA transpose per conv tile
    ([64,512] -> [128, 4, 64] 3D out) straight into the [128, 65]-per-chunk
    V' stationary layout (col 64 = ones for the softmax denominator row).
  - Attention per (mt, pair-of-2-chunks): 2 QK matmuls (f32r, k chunk
    [32,128] stationary) into a [128,1024] PSUM tile, one exp -> bf16,
    2 PV matmuls (bf16) accumulating E'^T in PSUM ([65, 512], row 64 =
    denominators). exp ops are split between ACT (exact, scale=1/EXPA)
    and DVE/Pool (2^x int16-bitcast approx; EXPA folded into w1).
  - mt0's attention interleaves with the KV conv, QK one conv tile behind,
    PV two behind (hides the K-epilogue and V-transpose latency). mt1..3
    run after with a QK/PV two-stage pipeline. PSUM: 2 conv + 4 st + 2 et.
  - Normalize tail split in column halves across DVE (recip, mul) and
    Pool (broadcast, add) to shrink the exposed critical path at the end.
  - Inputs ride 2 bundled small-weight DMAs + per-half chunked xb/xc
    (sync HWDGE queue for rows 0:64, gpsimd SWDGE for rows 64:128).
"""

import numpy as np

import concourse.bacc as bacc
import concourse.mybir as mybir
import concourse.tile as tile
from concourse import bass_utils
from concourse.masks import make_identity

F32 = mybir.dt.float32
F32R = mybir.dt.float32r
BF16 = mybir.dt.bfloat16
I16 = mybir.dt.int16
AF = mybir.ActivationFunctionType

B, C, CH, H, W = 4, 64, 32, 64, 64
N = H * W                     # 4096 keys
M = N // 2                    # 2048 query rows per core
NCORES = 8
XC_LEN = 4352                 # padded flat A1_C: 66*64+2 = 4226, padded up
XB_LEN = 2304                 # padded flat A1_B half: 34*64+2 = 2178, padded up
NKC = N // 128                # 32 key chunks
NPAIR = NKC // 2              # 16 chunk pairs (one [128,1024] st tile each)

# bund64 column offsets: wc1 | wc23 | ecl | ecr | ebl | ebr
B64_WC1, B64_WC23, B64_ECL, B64_ECR, B64_EBL, B64_EBR, B64_LEN = (
    0, 192, 960, 1026, 1093, 1127, 1162)
# bund128 column offsets: w1t | w23t | b23 | b1 | bv (V bias at rows 0:64)
B128_W1, B128_W23, B128_B23, B128_B1, B128_BV, B128_LEN = 0, 192, 960, 961, 962, 963

# 2^x trick constants: S_psum = EXPA * s (EXPA folded into w1 on host);
# p = exp(s) ~ bitcast_bf16(int16(S_psum + EXPB)).
EXPA = 128.0 / float(np.log(2.0))          # 184.664965...
EXPB_ROUND = 16256.0 - 5.5053              # round-to-nearest int16 convert
EXPB_TRUNC = EXPB_ROUND + 0.5              # truncate-toward-zero convert


def _exp_engine(mt, p):
    """exp engine for (mt, pair): ACT exact, or DVE 2^x approx."""
    if p % 3 == 2:
        return "dve"
    return "act"


_cache = {}


def _r32r(x):
    """Round fp32 -> float32r (zero low 12 mantissa bits, round to nearest)."""
    x = np.ascontiguousarray(x, np.float32)
    b = x.view(np.uint32).astype(np.uint64)
    out = (((b + np.uint64(1 << 11)) & np.uint64(0xFFFFF000)).astype(np.uint32)).view(np.float32)
    return np.ascontiguousarray(out)


def _build(dbg=False, expb=EXPB_TRUNC):
    nc = bacc.Bacc("TRN2", target_bir_lowering=False, debug=False)

    xc = nc.dram_tensor("xc", [128, XC_LEN], F32R, kind="ExternalInput")
    xb = nc.dram_tensor("xb", [128, XB_LEN], F32R, kind="ExternalInput")
    bund64 = nc.dram_tensor("bund64", [C, B64_LEN], F32R, kind="ExternalInput")
    bund128 = nc.dram_tensor("bund128", [128, B128_LEN], F32R, kind="ExternalInput")
    resid = nc.dram_tensor("resid", [C, M], F32, kind="ExternalInput")
    out_d = nc.dram_tensor("out", [C, M], F32, kind="ExternalOutput")
    if dbg:
        k_d = nc.dram_tensor("k_dbg", [CH, N], F32, kind="ExternalOutput")
        q_d = nc.dram_tensor("q_dbg", [CH, M], F32, kind="ExternalOutput")
        v_d = nc.dram_tensor("v_dbg", [128, NKC * 65], F32, kind="ExternalOutput")

    with tile.TileContext(nc) as tc:
        with (
            tc.tile_pool(name="big", bufs=1) as big,
            tc.tile_pool(name="work", bufs=2) as work,
            tc.tile_pool(name="expool", bufs=6) as expool,
        ):
            xc_sb = big.tile([128, XC_LEN], F32R, tag="xc")
            xb_sb = big.tile([128, XB_LEN], F32R, tag="xb")
            b64_sb = big.tile([C, B64_LEN], F32R, tag="b64")
            b128_sb = big.tile([128, B128_LEN], F32R, tag="b128")
            res_sb = big.tile([C, M], F32, tag="res")
            corrS = big.tile([C, 320], F32, tag="corrS")
            k_sb = big.tile([CH, N], F32R, tag="k")
            qt_sb = big.tile([CH, M], F32R, tag="qt")
            v_sb = big.tile([128, NKC * 65], BF16, tag="v")

            wc1_sb = b64_sb[:, B64_WC1:B64_WC1 + 6 * CH]
            wc23_sb = b64_sb[:, B64_WC23:B64_WC23 + 6 * 128]
            ecl_sb = b64_sb[:, B64_ECL:B64_ECL + 66]
            ecr_sb = b64_sb[:, B64_ECR:B64_ECR + 67]
            ebl_sb = b64_sb[:, B64_EBL:B64_EBL + 34]
            ebr_sb = b64_sb[:, B64_EBR:B64_EBR + 35]
            w1_sb = b128_sb[:, B128_W1:B128_W1 + 6 * CH]
            w23_sb = b128_sb[:, B128_W23:B128_W23 + 6 * 128]
            b23_sb = b128_sb[:, B128_B23:B128_B23 + 1].bitcast(F32)
            b1_sb = b128_sb[0:CH, B128_B1:B128_B1 + 1].bitcast(F32)
            bv_sb = b128_sb[0:C, B128_BV:B128_BV + 1].bitcast(F32)

            # ---- input DMA head ----
            # sync/HWDGE: bundles + rows 0:64 of xb/xc (chunked);
            # gpsimd/SWDGE (Pool desc-gen): rows 64:128 + resid.
            nc.sync.dma_start(out=b64_sb[:], in_=bund64.ap())
            nc.sync.dma_start(out=b128_sb[:], in_=bund128.ap())
            XBC = [0, 704, 1408, XB_LEN]
            for lo, hi in zip(XBC[:-1], XBC[1:]):
                nc.gpsimd.dma_start(out=xb_sb[C:128, lo:hi], in_=xb.ap()[C:128, lo:hi])
                nc.sync.dma_start(out=xb_sb[0:C, lo:hi], in_=xb.ap()[0:C, lo:hi])
            XCC = [0, 1152, 2304, 3456, XC_LEN]
            for lo, hi in zip(XCC[:-1], XCC[1:]):
                nc.gpsimd.dma_start(out=xc_sb[C:128, lo:hi], in_=xc.ap()[C:128, lo:hi])
                nc.sync.dma_start(out=xc_sb[0:C, lo:hi], in_=xc.ap()[0:C, lo:hi])

            # ones columns of V' (col 64 of each chunk) + exp table warm
            ones_bf = big.tile([128, NKC], BF16, tag="ones")
            nc.vector.memset(ones_bf[:], 1.0)
            v3 = v_sb[:].rearrange("p (a b) -> p a b", b=65)
            nc.vector.tensor_copy(v3[:, :, 64], ones_bf[:])
            warm = big.tile([128, 1], F32, tag="warm")
            nc.scalar.activation(warm[:], b23_sb[:], AF.Exp)

            # ---- edge-correction matmuls (own PSUM bank, freed early) ----
            # layout (SBUF partition-aligned with consumers):
            #   Q-L [0:32, 0:32]   Q-R [0:32, 32:64]
            #   K-L [0:32, 64:128] K-R [0:32, 128:192]
            #   V-L [0:64, 192:256] V-R [0:64, 256:320]
            with tc.tile_pool(name="vtp", bufs=1, space="PSUM") as vtp:
                corrT = vtp.tile([C, 320], F32, tag="corrT")
                mms = []
                for dy in range(3):
                    # V first: the group's start zero-region must span the
                    # full partition range (0:64) of the tile.
                    mms.append((corrT[0:C, 192:256],
                                wc23_sb[:, (2 * dy) * 128 + 64:(2 * dy) * 128 + 128],
                                ecl_sb[:, dy:dy + 64]))
                    mms.append((corrT[0:C, 256:320],
                                wc23_sb[:, (2 * dy + 1) * 128 + 64:(2 * dy + 1) * 128 + 128],
                                ecr_sb[:, dy + 1:dy + 65]))
                    mms.append((corrT[0:CH, 0:32],
                                wc1_sb[:, (2 * dy) * CH:(2 * dy + 1) * CH],
                                ebl_sb[:, dy:dy + 32]))
                    mms.append((corrT[0:CH, 32:64],
                                wc1_sb[:, (2 * dy + 1) * CH:(2 * dy + 2) * CH],
                                ebr_sb[:, dy + 1:dy + 33]))
                    mms.append((corrT[0:CH, 64:128],
                                wc23_sb[:, (2 * dy) * 128:(2 * dy) * 128 + CH],
                                ecl_sb[:, dy:dy + 64]))
                    mms.append((corrT[0:CH, 128:192],
                                wc23_sb[:, (2 * dy + 1) * 128:(2 * dy + 1) * 128 + CH],
                                ecr_sb[:, dy + 1:dy + 65]))
                # first and last matmul must span the full 0:64 partition
                # range: start's zero region and stop's group close both
                # cover only the instruction's own partitions.
                mms = [mms[0]] + mms[2:] + [mms[1]]
                for i_mm, (o, l, r) in enumerate(mms):
                    nc.tensor.matmul(o, l.bitcast(F32), r.bitcast(F32),
                                     start=(i_mm == 0), stop=(i_mm == len(mms) - 1))
                nc.vector.tensor_copy(corrS[0:CH, 0:192], corrT[0:CH, 0:192])
                nc.vector.tensor_copy(corrS[0:C, 192:320], corrT[0:C, 192:320])
            corrQ = corrS[0:CH, 0:64]

            def conv_tile(pk, x_sb, w_sb, j, co):
                for dx in range(3):
                    base = j * 512 + dx
                    nc.tensor.matmul(
                        pk[:], w_sb[:, dx * co:(dx + 1) * co],
                        x_sb[:, base:base + 512],
                        start=(dx == 0), stop=False,
                    )
                for dx in range(3):
                    base = j * 512 + 2 * 64 + dx
                    nc.tensor.matmul(
                        pk[:], w_sb[0:C, (3 + dx) * co:(4 + dx) * co],
                        x_sb[0:C, base:base + 512],
                        start=False, stop=(dx == 2),
                    )

            # ---- conv + attention ----
            identb = big.tile([C, C], BF16, tag="identb")
            make_identity(nc, identb[:])

            with tc.tile_pool(name="eps", bufs=2, space="PSUM") as eps:
                ets = {}

                def attn_qk(mt, p, pool):
                    st = pool.tile([128, 1024], F32, tag="st")
                    for i in range(2):
                        kk = 2 * p + i
                        nc.tensor.matmul(
                            st[:, i * 512:(i + 1) * 512],
                            k_sb[:, kk * 128:(kk + 1) * 128],
                            qt_sb[:, mt * 512:(mt + 1) * 512],
                            start=True, stop=True,
                        )
                    ex = expool.tile([128, 1024], BF16, tag="ex")
                    if _exp_engine(mt, p) == "act":
                        nc.scalar.activation(ex[:], st[:], AF.Exp, scale=1.0 / EXPA)
                    else:
                        nc.vector.tensor_scalar_add(ex[:].bitcast(I16), st[:], expb)
                    return ex

                def attn_pv(mt, p, ex):
                    for i in range(2):
                        kk = 2 * p + i
                        nc.tensor.matmul(
                            ets[mt][0:C + 1, :],
                            v_sb[:, kk * 65:kk * 65 + 65],
                            ex[:, i * 512:(i + 1) * 512],
                            start=(kk == 0), stop=(kk == NKC - 1),
                        )

                def attn_tail(mt):
                    et = ets[mt]
                    sls = [slice(h * 256, (h + 1) * 256) for h in range(2)]
                    osls = [slice(mt * 512 + h * 256, mt * 512 + (h + 1) * 256)
                            for h in range(2)]
                    ots = []
                    bcs = []
                    for h in range(2):
                        recip = work.tile([1, 256], F32, tag="recip")
                        nc.vector.reciprocal(recip[:], et[C:C + 1, sls[h]])
                        bc = work.tile([C, 256], F32, tag="bc")
                        nc.gpsimd.partition_broadcast(bc[:], recip[:])
                        bcs.append(bc)
                    for h in range(2):
                        ot = work.tile([C, 256], F32, tag="ot")
                        nc.vector.tensor_mul(ot[:], et[0:C, sls[h]], bcs[h][:])
                        ots.append(ot)
                    # h0's residual add on Pool, h1's on DVE: the two finish
                    # in parallel right behind the muls
                    nc.gpsimd.tensor_add(ots[0][:], ots[0][:], res_sb[:, osls[0]])
                    nc.sync.dma_start(out=out_d.ap()[:, osls[0]], in_=ots[0][:])
                    nc.vector.tensor_add(ots[1][:], ots[1][:], res_sb[:, osls[1]])
                    nc.sync.dma_start(out=out_d.ap()[:, osls[1]], in_=ots[1][:])

                # ---- conv phase (attention PSUM not yet open) ----
                with (
                    tc.tile_pool(name="cps", bufs=3, space="PSUM") as cps,
                    tc.tile_pool(name="vtp2", bufs=2, space="PSUM") as vtp2,
                ):
                    def q_conv_tile(j):
                        pq0 = cps.tile([128, 512], F32, tag="cv")
                        pq = pq0[0:CH, :]
                        conv_tile(pq, xb_sb, w1_sb, j, CH)
                        qsl = qt_sb[:, j * 512:(j + 1) * 512]
                        nc.vector.tensor_scalar_add(qsl, pq[:], b1_sb)
                        q3 = qsl.rearrange("p (h w) -> p h w", w=64)
                        nc.vector.tensor_add(q3[:, :, 0], q3[:, :, 0], corrQ[:, j * 8:(j + 1) * 8])
                        nc.vector.tensor_add(q3[:, :, 63], q3[:, :, 63], corrQ[:, 32 + j * 8:32 + (j + 1) * 8])

                    def kv_conv_tile(j):
                        pk = cps.tile([128, 512], F32, tag="cv")
                        conv_tile(pk, xc_sb, w23_sb, j, 128)
                        ksl = k_sb[:, j * 512:(j + 1) * 512]
                        nc.vector.tensor_scalar_add(ksl, pk[0:CH, :], b23_sb[0:CH, :])
                        k3 = ksl.rearrange("p (h w) -> p h w", w=64)
                        nc.gpsimd.tensor_add(k3[:, :, 0], k3[:, :, 0], corrS[0:CH, 64 + j * 8:64 + (j + 1) * 8])
                        nc.gpsimd.tensor_add(k3[:, :, 63], k3[:, :, 63], corrS[0:CH, 128 + j * 8:128 + (j + 1) * 8])
                        vtmp = work.tile([C, 512], BF16, tag="vtmp")
                        nc.scalar.activation(vtmp[:], pk[64:128, :], AF.Identity, bias=bv_sb)
                        v3t = vtmp[:].rearrange("p (h w) -> p h w", w=64)
                        nc.vector.tensor_add(v3t[:, :, 0], v3t[:, :, 0], corrS[0:C, 192 + j * 8:192 + (j + 1) * 8])
                        nc.vector.tensor_add(v3t[:, :, 63], v3t[:, :, 63], corrS[0:C, 256 + j * 8:256 + (j + 1) * 8])
                        return vtmp

                    def v_transpose(j, vtmp):
                        vt = vtp2.tile([128, 256], BF16, tag="vt")
                        for c4 in range(4):
                            nc.tensor.transpose(
                                vt[:, c4 * 64:(c4 + 1) * 64],
                                vtmp[:, c4 * 128:(c4 + 1) * 128], identb[:])
                        vslc = v_sb[:, j * 260:(j + 1) * 260]
                        vdst = vslc.rearrange("p (c f) -> p c f", f=65)[:, :, 0:64]
                        nc.scalar.activation(vdst, vt[:].rearrange("p (c f) -> p c f", f=64),
                                             AF.Identity)

                    for j in range(4):
                        q_conv_tile(j)
                    vtmps = {}
                    for j in range(8):
                        vtmps[j] = kv_conv_tile(j)
                        if j >= 2:
                            v_transpose(j - 2, vtmps.pop(j - 2))
                    for j in (6, 7):
                        v_transpose(j, vtmps.pop(j))
                    nc.gpsimd.dma_start(out=res_sb[:], in_=resid.ap())

                if dbg:
                    nc.sync.dma_start(out=k_d.ap(), in_=k_sb[:].bitcast(F32))
                    nc.sync.dma_start(out=q_d.ap(), in_=qt_sb[:].bitcast(F32))
                    vdbg = big.tile([128, NKC * 65], F32, tag="vdbg")
                    nc.vector.tensor_copy(vdbg[:], v_sb[:])
                    nc.sync.dma_start(out=v_d.ap(), in_=vdbg[:])

                # ---- attention phase: all 4 mts, three-stage pipeline ----
                with tc.tile_pool(name="aps2", bufs=3, space="PSUM") as aps2:
                    for mt in range(4):
                        etm = eps.tile([128, 512], F32, tag="et")
                        ets[mt] = etm
                        pend = []
                        for p in range(NPAIR):
                            pend.append((p, attn_qk(mt, p, aps2)))
                            if p == 1 and mt > 0:
                                attn_tail(mt - 1)
                            if len(pend) > 2:
                                pp, pex = pend.pop(0)
                                attn_pv(mt, pp, pex)
                        for pp, pex in pend:
                            attn_pv(mt, pp, pex)
                    attn_tail(3)

    nc.compile()
    return nc


def _prep_core_inputs(inputs, core):
    A1_B = np.asarray(inputs["A1_B"], np.float32)
    A1_C = np.asarray(inputs["A1_C"], np.float32)
    w1 = np.asarray(inputs["w1"], np.float32) * EXPA
    b1 = np.asarray(inputs["b1"], np.float32) * EXPA
    w2 = np.asarray(inputs["w2"], np.float32)
    b2 = np.asarray(inputs["b2"], np.float32)
    w3 = np.asarray(inputs["w3"], np.float32)
    b3 = np.asarray(inputs["b3"], np.float32)
    b = core // 2
    half = core % 2
    h0 = half * 32

    xc = np.zeros((128, XC_LEN), np.float32)
    flat_c = np.zeros((C, H + 2, W), np.float32)
    flat_c[:, 1:H + 1, :] = A1_C[b]
    xc[0:C, 1:1 + (H + 2) * W] = flat_c.reshape(C, -1)
    xc[C:128, 0:XC_LEN - 64] = xc[0:C, 64:XC_LEN]

    xb = np.zeros((128, XB_LEN), np.float32)
    flat_b = np.zeros((C, 34, W), np.float32)
    glo = h0 - 1
    src_lo = max(glo, 0)
    src_hi = min(h0 + 33, H)
    flat_b[:, src_lo - glo: src_hi - glo, :] = A1_B[b][:, src_lo:src_hi, :]
    xb[0:C, 1:1 + 34 * W] = flat_b.reshape(C, -1)
    xb[C:128, 0:XB_LEN - 64] = xb[0:C, 64:XB_LEN]

    # Tap layout: cols t*co.. with t=0..2 the (dy=0,dy=1) stacked pairs
    # (contraction rows 0:64 = dy0, 64:128 = dy1), t=3..5 the dy=2 singles.
    w1t = np.zeros((128, 6 * CH), np.float32)
    w23t = np.zeros((128, 6 * 128), np.float32)
    for dx in range(3):
        w1t[0:C, dx * CH:(dx + 1) * CH] = w1[:, :, 0, dx].T
        w1t[C:128, dx * CH:(dx + 1) * CH] = w1[:, :, 1, dx].T
        w1t[0:C, (3 + dx) * CH:(4 + dx) * CH] = w1[:, :, 2, dx].T
        w23t[0:C, dx * 128: dx * 128 + CH] = w2[:, :, 0, dx].T
        w23t[C:128, dx * 128: dx * 128 + CH] = w2[:, :, 1, dx].T
        w23t[0:C, dx * 128 + 64: dx * 128 + 128] = w3[:, :, 0, dx].T
        w23t[C:128, dx * 128 + 64: dx * 128 + 128] = w3[:, :, 1, dx].T
        w23t[0:C, (3 + dx) * 128: (3 + dx) * 128 + CH] = w2[:, :, 2, dx].T
        w23t[0:C, (3 + dx) * 128 + 64: (3 + dx) * 128 + 128] = w3[:, :, 2, dx].T
    wc1 = np.zeros((C, 6 * CH), np.float32)
    wc23 = np.zeros((C, 6 * 128), np.float32)
    for dy in range(3):
        for side, dx in ((0, 0), (1, 2)):
            i = 2 * dy + side
            wc1[:, i * CH:(i + 1) * CH] = -w1[:, :, dy, dx].T
            wc23[:, i * 128: i * 128 + CH] = -w2[:, :, dy, dx].T
            wc23[:, i * 128 + 64: i * 128 + 128] = -w3[:, :, dy, dx].T

    xcr = _r32r(xc)
    xbr = _r32r(xb)

    bund64 = np.zeros((C, B64_LEN), np.float32)
    bund64[:, B64_WC1:B64_WC1 + 6 * CH] = _r32r(wc1)
    bund64[:, B64_WC23:B64_WC23 + 6 * 128] = _r32r(wc23)
    bund64[:, B64_ECL:B64_ECL + 66] = xcr[0:C, (np.arange(66)) * 64]
    bund64[:, B64_ECR:B64_ECR + 67] = xcr[0:C, (np.arange(67)) * 64 + 1]
    bund64[:, B64_EBL:B64_EBL + 34] = xbr[0:C, (np.arange(34)) * 64]
    bund64[:, B64_EBR:B64_EBR + 35] = xbr[0:C, (np.arange(35)) * 64 + 1]

    bund128 = np.zeros((128, B128_LEN), np.float32)
    bund128[:, B128_W1:B128_W1 + 6 * CH] = _r32r(w1t)
    bund128[:, B128_W23:B128_W23 + 6 * 128] = _r32r(w23t)
    bund128[:, B128_B23] = np.concatenate([b2, np.zeros(32, np.float32), b3])
    bund128[0:CH, B128_B1] = b1
    bund128[0:C, B128_BV] = b3

    resid = np.ascontiguousarray(A1_C[b][:, h0:h0 + 32, :].reshape(C, M))
    return {
        "xc": xcr,
        "xb": xbr,
        "bund64": bund64,
        "bund128": bund128,
        "resid": resid,
    }


def _run(inputs, trace=False, dbg=False):
    key = ("nc", dbg)
    if key not in _cache:
        _cache[key] = _build(dbg)
    nc = _cache[key]
    in_maps = [_prep_core_inputs(inputs, i) for i in range(NCORES)]
    res = bass_utils.run_bass_kernel_spmd(
        nc, in_maps, core_ids=list(range(NCORES)), trace=trace)
    out = np.empty((B, C, H, W), np.float32)
    for i in range(NCORES):
        b, half = i // 2, i % 2
        out[b, :, half * 32:half * 32 + 32, :] = res.results[i]["out"].reshape(C, 32, W)
    return out, res


def kernel(**inputs):
    out, _ = _run(inputs, trace=False)
    return out


# revision 25
# speedup vs baseline: 1.1973x; 1.0184x over previous
"""Trainium2 Bass kernel for nn_BSAM_60129542251.

Conv-QKV self-attention block (B=4, C=64, H=W=64):
  Q = conv3x3(A1_B, w1)  -> [b, 32, 4096]
  K = conv3x3(A1_C, w2)  -> [b, 32, 4096]
  V = conv3x3(A1_C, w3)  -> [b, 64, 4096]
  E = softmax(Q^T K) V^T -> [b, 4096, 64];  out = E^T + A1_C

Sharding: 8 cores; core i handles sample b=i//2, row-half i%2 (2048 query
rows). K/V convs are duplicated within a sample pair; Q conv runs on the
core's half only. Attention is fully fused on-chip (no S matrix in HBM).

v3 structure (engine-balanced, conv/attention interleaved, software
pipelined):
  - Conv = 9 shifted matmuls done as 6 (dy-pairs stacked in contraction;
    rows 64:128 of the flat input hold a 64-shifted copy), with compact
    edge-correction matmuls cancelling the w=0/w=63 row-wrap reads.
  - Conv epilogues (PSUM->SBUF + bias) run on Pool (K, Q as fp32 bits into
    f32r tiles) and DVE (V -> bf16), keeping ACT free for exp.
  - V tiles transposed via one batched XBAR DMA transpose per conv tile
    ([64,512] -> [128, 4, 64] 3D out) straight into the [128, 65]-per-chunk
    V' stationary layout (col 64 = ones for the softmax denominator row).
  - Attention per (mt, pair-of-2-chunks): 2 QK matmuls (f32r, k chunk
    [32,128] stationary) into a [128,1024] PSUM tile, one exp -> bf16,
    2 PV matmuls (bf16) accumulating E'^T in PSUM ([65, 512], row 64 =
    denominators). exp ops are split between ACT (exact, scale=1/EXPA)
    and DVE/Pool (2^x int16-bitcast approx; EXPA folded into w1).
  - mt0's attention interleaves with the KV conv, QK one conv tile behind,
    PV two behind (hides the K-epilogue and V-transpose latency). mt1..3
    run after with a QK/PV two-stage pipeline. PSUM: 2 conv + 4 st + 2 et.
  - Normalize tail split in column halves across DVE (recip, mul) and
    Pool (broadcast, add) to shrink the exposed critical path at the end.
  - Inputs ride 2 bundled small-weight DMAs + per-half chunked xb/xc
    (sync HWDGE queue for rows 0:64, gpsimd SWDGE for rows 64:128).
"""

import numpy as np

import concourse.bacc as bacc
import concourse.mybir as mybir
import concourse.tile as tile
from concourse import bass_utils
from concourse.masks import make_identity

F32 = mybir.dt.float32
F32R = mybir.dt.float32r
BF16 = mybir.dt.bfloat16
I16 = mybir.dt.int16
AF = mybir.ActivationFunctionType

B, C, CH, H, W = 4, 64, 32, 64, 64
N = H * W                     # 4096 keys
M = N // 2                    # 2048 query rows per core
NCORES = 8
XC_LEN = 4352                 # padded flat A1_C: 66*64+2 = 4226, padded up
XB_LEN = 2304                 # padded flat A1_B half: 34*64+2 = 2178, padded up
NKC = N // 128                # 32 key chunks
NPAIR = NKC // 2              # 16 chunk pairs (one [128,1024] st tile each)

# bund64 column offsets: wc1 | wc23 | ecl | ecr | ebl | ebr
B64_WC1, B64_WC23, B64_ECL, B64_ECR, B64_EBL, B64_EBR, B64_LEN = (
    0, 192, 960, 1026, 1093, 1127, 1162)
# bund128 column offsets: w1t | w23t | b23 | b1 | bv (V bias at rows 0:64)
B128_W1, B128_W23, B128_B23, B128_B1, B128_BV, B128_LEN = 0, 192, 960, 961, 962, 963

# 2^x trick constants: S_psum = EXPA * s (EXPA folded into w1 on host);
# p = exp(s) ~ bitcast_bf16(int16(S_psum + EXPB)).
EXPA = 128.0 / float(np.log(2.0))          # 184.664965...
EXPB_ROUND = 16256.0 - 5.5053              # round-to-nearest int16 convert
EXPB_TRUNC = EXPB_ROUND + 0.5              # truncate-toward-zero convert


def _exp_engine(mt, p):
    """exp engine for (mt, pair): ACT exact, or DVE 2^x approx."""
    if p % 3 == 2:
        return "dve"
    return "act"


_cache = {}


def _r32r(x):
    """Round fp32 -> float32r (zero low 12 mantissa bits, round to nearest)."""
    x = np.ascontiguousarray(x, np.float32)
    b = x.view(np.uint32).astype(np.uint64)
    out = (((b + np.uint64(1 << 11)) & np.uint64(0xFFFFF000)).astype(np.uint32)).view(np.float32)
    return np.ascontiguousarray(out)


def _build(dbg=False, expb=EXPB_TRUNC):
    nc = bacc.Bacc("TRN2", target_bir_lowering=False, debug=False)

    xc = nc.dram_tensor("xc", [128, XC_LEN], F32R, kind="ExternalInput")
    xb = nc.dram_tensor("xb", [128, XB_LEN], F32R, kind="ExternalInput")
    bund64 = nc.dram_tensor("bund64", [C, B64_LEN], F32R, kind="ExternalInput")
    bund128 = nc.dram_tensor("bund128", [128, B128_LEN], F32R, kind="ExternalInput")
    resid = nc.dram_tensor("resid", [C, M], F32, kind="ExternalInput")
    out_d = nc.dram_tensor("out", [C, M], F32, kind="ExternalOutput")
    if dbg:
        k_d = nc.dram_tensor("k_dbg", [CH, N], F32, kind="ExternalOutput")
        q_d = nc.dram_tensor("q_dbg", [CH, M], F32, kind="ExternalOutput")
        v_d = nc.dram_tensor("v_dbg", [128, NKC * 65], F32, kind="ExternalOutput")

    with tile.TileContext(nc) as tc:
        with (
            tc.tile_pool(name="big", bufs=1) as big,
            tc.tile_pool(name="work", bufs=2) as work,
            tc.tile_pool(name="expool", bufs=6) as expool,
        ):
            xc_sb = big.tile([128, XC_LEN], F32R, tag="xc")
            xb_sb = big.tile([128, XB_LEN], F32R, tag="xb")
            b64_sb = big.tile([C, B64_LEN], F32R, tag="b64")
            b128_sb = big.tile([128, B128_LEN], F32R, tag="b128")
            res_sb = big.tile([C, M], F32, tag="res")
            corrS = big.tile([C, 320], F32, tag="corrS")
            k_sb = big.tile([CH, N], F32R, tag="k")
            qt_sb = big.tile([CH, M], F32R, tag="qt")
            v_sb = big.tile([128, NKC * 65], BF16, tag="v")

            wc1_sb = b64_sb[:, B64_WC1:B64_WC1 + 6 * CH]
            wc23_sb = b64_sb[:, B64_WC23:B64_WC23 + 6 * 128]
            ecl_sb = b64_sb[:, B64_ECL:B64_ECL + 66]
            ecr_sb = b64_sb[:, B64_ECR:B64_ECR + 67]
            ebl_sb = b64_sb[:, B64_EBL:B64_EBL + 34]
            ebr_sb = b64_sb[:, B64_EBR:B64_EBR + 35]
            w1_sb = b128_sb[:, B128_W1:B128_W1 + 6 * CH]
            w23_sb = b128_sb[:, B128_W23:B128_W23 + 6 * 128]
            b23_sb = b128_sb[:, B128_B23:B128_B23 + 1].bitcast(F32)
            b1_sb = b128_sb[0:CH, B128_B1:B128_B1 + 1].bitcast(F32)
            bv_sb = b128_sb[0:C, B128_BV:B128_BV + 1].bitcast(F32)

            # ---- input DMA head ----
            # sync/HWDGE: bundles + rows 0:64 of xb/xc (chunked);
            # gpsimd/SWDGE (Pool desc-gen): rows 64:128 + resid.
            nc.sync.dma_start(out=b64_sb[:], in_=bund64.ap())
            nc.sync.dma_start(out=b128_sb[:], in_=bund128.ap())
            XBC = [0, 704, 1408, XB_LEN]
            for lo, hi in zip(XBC[:-1], XBC[1:]):
                nc.gpsimd.dma_start(out=xb_sb[C:128, lo:hi], in_=xb.ap()[C:128, lo:hi])
                nc.sync.dma_start(out=xb_sb[0:C, lo:hi], in_=xb.ap()[0:C, lo:hi])
            XCC = [0, 1152, 2304, 3456, XC_LEN]
            for lo, hi in zip(XCC[:-1], XCC[1:]):
                nc.gpsimd.dma_start(out=xc_sb[C:128, lo:hi], in_=xc.ap()[C:128, lo:hi])
                nc.sync.dma_start(out=xc_sb[0:C, lo:hi], in_=xc.ap()[0:C, lo:hi])

            # ones columns of V' (col 64 of each chunk) + exp table warm
            ones_bf = big.tile([128, NKC], BF16, tag="ones")
            nc.vector.memset(ones_bf[:], 1.0)
            v3 = v_sb[:].rearrange("p (a b) -> p a b", b=65)
            nc.vector.tensor_copy(v3[:, :, 64], ones_bf[:])
            warm = big.tile([128, 1], F32, tag="warm")
            nc.scalar.activation(warm[:], b23_sb[:], AF.Exp)

            # ---- edge-correction matmuls (own PSUM bank, freed early) ----
            # layout (SBUF partition-aligned with consumers):
            #   Q-L [0:32, 0:32]   Q-R [0:32, 32:64]
            #   K-L [0:32, 64:128] K-R [0:32, 128:192]
            #   V-L [0:64, 192:256] V-R [0:64, 256:320]
            with tc.tile_pool(name="vtp", bufs=1, space="PSUM") as vtp:
                corrT = vtp.tile([C, 320], F32, tag="corrT")
                mms = []
                for dy in range(3):
                    # V first: the group's start zero-region must span the
                    # full partition range (0:64) of the tile.
                    mms.append((corrT[0:C, 192:256],
                                wc23_sb[:, (2 * dy) * 128 + 64:(2 * dy) * 128 + 128],
                                ecl_sb[:, dy:dy + 64]))
                    mms.append((corrT[0:C, 256:320],
                                wc23_sb[:, (2 * dy + 1) * 128 + 64:(2 * dy + 1) * 128 + 128],
                                ecr_sb[:, dy + 1:dy + 65]))
                    mms.append((corrT[0:CH, 0:32],
                                wc1_sb[:, (2 * dy) * CH:(2 * dy + 1) * CH],
                                ebl_sb[:, dy:dy + 32]))
                    mms.append((corrT[0:CH, 32:64],
                                wc1_sb[:, (2 * dy + 1) * CH:(2 * dy + 2) * CH],
                                ebr_sb[:, dy + 1:dy + 33]))
                    mms.append((corrT[0:CH, 64:128],
                                wc23_sb[:, (2 * dy) * 128:(2 * dy) * 128 + CH],
                                ecl_sb[:, dy:dy + 64]))
                    mms.append((corrT[0:CH, 128:192],
                                wc23_sb[:, (2 * dy + 1) * 128:(2 * dy + 1) * 128 + CH],
                                ecr_sb[:, dy + 1:dy + 65]))
                # first and last matmul must span the full 0:64 partition
                # range: start's zero region and stop's group close both
                # cover only the instruction's own partitions.
                mms = [mms[0]] + mms[2:] + [mms[1]]
                for i_mm, (o, l, r) in enumerate(mms):
                    nc.tensor.matmul(o, l.bitcast(F32), r.bitcast(F32),
                                     start=(i_mm == 0), stop=(i_mm == len(mms) - 1))
                nc.vector.tensor_copy(corrS[0:CH, 0:192], corrT[0:CH, 0:192])
                nc.vector.tensor_copy(corrS[0:C, 192:320], corrT[0:C, 192:320])
            corrQ = corrS[0:CH, 0:64]

            def conv_tile(pk, x_sb, w_sb, j, co):
                for dx in range(3):
                    base = j * 512 + dx
                    nc.tensor.matmul(
                        pk[:], w_sb[:, dx * co:(dx + 1) * co],
                        x_sb[:, base:base + 512],
                        start=(dx == 0), stop=False,
                    )
                for dx in range(3):
                    base = j * 512 + 2 * 64 + dx
                    nc.tensor.matmul(
                        pk[:], w_sb[0:C, (3 + dx) * co:(4 + dx) * co],
                        x_sb[0:C, base:base + 512],
                        start=False, stop=(dx == 2),
                    )

            # ---- conv + attention ----
            identb = big.tile([C, C], BF16, tag="identb")
            make_identity(nc, identb[:])

            with tc.tile_pool(name="eps", bufs=2, space="PSUM") as eps:
                ets = {}

                def attn_qk(mt, p, pool):
                    st = pool.tile([128, 1024], F32, tag="st")
                    for i in range(2):
                        kk = 2 * p + i
                        nc.tensor.matmul(
                            st[:, i * 512:(i + 1) * 512],
                            k_sb[:, kk * 128:(kk + 1) * 128],
                            qt_sb[:, mt * 512:(mt + 1) * 512],
                            start=True, stop=True,
                        )
                    ex = expool.tile([128, 1024], BF16, tag="ex")
                    if _exp_engine(mt, p) == "act":
                        nc.scalar.activation(ex[:], st[:], AF.Exp, scale=1.0 / EXPA)
                    else:
                        nc.vector.tensor_scalar_add(ex[:].bitcast(I16), st[:], expb)
                    return ex

                def attn_pv(mt, p, ex):
                    for i in range(2):
                        kk = 2 * p + i
                        nc.tensor.matmul(
                            ets[mt][0:C + 1, :],
                            v_sb[:, kk * 65:kk * 65 + 65],
                            ex[:, i * 512:(i + 1) * 512],
                            start=(kk == 0), stop=(kk == NKC - 1),
                        )

                def attn_tail(mt):
                    et = ets[mt]
                    sls = [slice(h * 256, (h + 1) * 256) for h in range(2)]
                    osls = [slice(mt * 512 + h * 256, mt * 512 + (h + 1) * 256)
                            for h in range(2)]
                    ots = []
                    bcs = []
                    for h in range(2):
                        recip = work.tile([1, 256], F32, tag="recip")
                        nc.vector.reciprocal(recip[:], et[C:C + 1, sls[h]])
                        bc = work.tile([C, 256], F32, tag="bc")
                        nc.gpsimd.partition_broadcast(bc[:], recip[:])
                        bcs.append(bc)
                    for h in range(2):
                        ot = work.tile([C, 256], F32, tag="ot")
                        nc.vector.tensor_mul(ot[:], et[0:C, sls[h]], bcs[h][:])
                        ots.append(ot)
                    # h0's residual add on Pool, h1's on DVE: the two finish
                    # in parallel right behind the muls
                    nc.gpsimd.tensor_add(ots[0][:], ots[0][:], res_sb[:, osls[0]])
                    nc.sync.dma_start(out=out_d.ap()[:, osls[0]], in_=ots[0][:])
                    nc.vector.tensor_add(ots[1][:], ots[1][:], res_sb[:, osls[1]])
                    nc.sync.dma_start(out=out_d.ap()[:, osls[1]], in_=ots[1][:])

                # ---- conv phase (attention PSUM not yet open) ----
                with (
                    tc.tile_pool(name="cps", bufs=3, space="PSUM") as cps,
                    tc.tile_pool(name="vtp2", bufs=2, space="PSUM") as vtp2,
                ):
                    def q_conv_tile(j):
                        pq0 = cps.tile([128, 512], F32, tag="cv")
                        pq = pq0[0:CH, :]
                        conv_tile(pq, xb_sb, w1_sb, j, CH)
                        qsl = qt_sb[:, j * 512:(j + 1) * 512]
                        nc.vector.tensor_scalar_add(qsl, pq[:], b1_sb)
                        q3 = qsl.rearrange("p (h w) -> p h w", w=64)
                        nc.vector.tensor_add(q3[:, :, 0], q3[:, :, 0], corrQ[:, j * 8:(j + 1) * 8])
                        nc.vector.tensor_add(q3[:, :, 63], q3[:, :, 63], corrQ[:, 32 + j * 8:32 + (j + 1) * 8])

                    def kv_conv_tile(j):
                        pk = cps.tile([128, 512], F32, tag="cv")
                        conv_tile(pk, xc_sb, w23_sb, j, 128)
                        ksl = k_sb[:, j * 512:(j + 1) * 512]
                        nc.vector.tensor_scalar_add(ksl, pk[0:CH, :], b23_sb[0:CH, :])
                        k3 = ksl.rearrange("p (h w) -> p h w", w=64)
                        nc.gpsimd.tensor_add(k3[:, :, 0], k3[:, :, 0], corrS[0:CH, 64 + j * 8:64 + (j + 1) * 8])
                        nc.gpsimd.tensor_add(k3[:, :, 63], k3[:, :, 63], corrS[0:CH, 128 + j * 8:128 + (j + 1) * 8])
                        vtmp = work.tile([C, 512], BF16, tag="vtmp")
                        nc.scalar.activation(vtmp[:], pk[64:128, :], AF.Identity, bias=bv_sb)
                        v3t = vtmp[:].rearrange("p (h w) -> p h w", w=64)
                        nc.vector.tensor_add(v3t[:, :, 0], v3t[:, :, 0], corrS[0:C, 192 + j * 8:192 + (j + 1) * 8])
                        nc.vector.tensor_add(v3t[:, :, 63], v3t[:, :, 63], corrS[0:C, 256 + j * 8:256 + (j + 1) * 8])
                        return vtmp

                    def v_transpose(j, vtmp):
                        vt = vtp2.tile([128, 256], BF16, tag="vt")
                        for c4 in range(4):
                            nc.tensor.transpose(
                                vt[:, c4 * 64:(c4 + 1) * 64],
                                vtmp[:, c4 * 128:(c4 + 1) * 128], identb[:])
                        vslc = v_sb[:, j * 260:(j + 1) * 260]
                        vdst = vslc.rearrange("p (c f) -> p c f", f=65)[:, :, 0:64]
                        nc.scalar.activation(vdst, vt[:].rearrange("p (c f) -> p c f", f=64),
                                             AF.Identity)

                    q_conv_tile(0)
                    vtmps = {}
                    for j in range(8):
                        vtmps[j] = kv_conv_tile(j)
                        if j >= 2:
                            v_transpose(j - 2, vtmps.pop(j - 2))
                    v_transpose(6, vtmps.pop(6))
                    q_conv_tile(1)
                    v_transpose(7, vtmps.pop(7))
                    for j in (2, 3):
                        q_conv_tile(j)
                    nc.gpsimd.dma_start(out=res_sb[:], in_=resid.ap())

                if dbg:
                    nc.sync.dma_start(out=k_d.ap(), in_=k_sb[:].bitcast(F32))
                    nc.sync.dma_start(out=q_d.ap(), in_=qt_sb[:].bitcast(F32))
                    vdbg = big.tile([128, NKC * 65], F32, tag="vdbg")
                    nc.vector.tensor_copy(vdbg[:], v_sb[:])
                    nc.sync.dma_start(out=v_d.ap(), in_=vdbg[:])

                # ---- attention phase: all 4 mts, three-stage pipeline ----
                with tc.tile_pool(name="aps2", bufs=3, space="PSUM") as aps2:
                    for mt in range(4):
                        etm = eps.tile([128, 512], F32, tag="et")
                        ets[mt] = etm
                        pend = []
                        for p in range(NPAIR):
                            pend.append((p, attn_qk(mt, p, aps2)))
                            if p == 1 and mt > 0:
                                attn_tail(mt - 1)
                            if len(pend) > 2:
                                pp, pex = pend.pop(0)
                                attn_pv(mt, pp, pex)
                        for pp, pex in pend:
                            attn_pv(mt, pp, pex)
                    attn_tail(3)

    nc.compile()
    return nc


def _prep_core_inputs(inputs, core):
    A1_B = np.asarray(inputs["A1_B"], np.float32)
    A1_C = np.asarray(inputs["A1_C"], np.float32)
    w1 = np.asarray(inputs["w1"], np.float32) * EXPA
    b1 = np.asarray(inputs["b1"], np.float32) * EXPA
    w2 = np.asarray(inputs["w2"], np.float32)
    b2 = np.asarray(inputs["b2"], np.float32)
    w3 = np.asarray(inputs["w3"], np.float32)
    b3 = np.asarray(inputs["b3"], np.float32)
    b = core // 2
    half = core % 2
    h0 = half * 32

    xc = np.zeros((128, XC_LEN), np.float32)
    flat_c = np.zeros((C, H + 2, W), np.float32)
    flat_c[:, 1:H + 1, :] = A1_C[b]
    xc[0:C, 1:1 + (H + 2) * W] = flat_c.reshape(C, -1)
    xc[C:128, 0:XC_LEN - 64] = xc[0:C, 64:XC_LEN]

    xb = np.zeros((128, XB_LEN), np.float32)
    flat_b = np.zeros((C, 34, W), np.float32)
    glo = h0 - 1
    src_lo = max(glo, 0)
    src_hi = min(h0 + 33, H)
    flat_b[:, src_lo - glo: src_hi - glo, :] = A1_B[b][:, src_lo:src_hi, :]
    xb[0:C, 1:1 + 34 * W] = flat_b.reshape(C, -1)
    xb[C:128, 0:XB_LEN - 64] = xb[0:C, 64:XB_LEN]

    # Tap layout: cols t*co.. with t=0..2 the (dy=0,dy=1) stacked pairs
    # (contraction rows 0:64 = dy0, 64:128 = dy1), t=3..5 the dy=2 singles.
    w1t = np.zeros((128, 6 * CH), np.float32)
    w23t = np.zeros((128, 6 * 128), np.float32)
    for dx in range(3):
        w1t[0:C, dx * CH:(dx + 1) * CH] = w1[:, :, 0, dx].T
        w1t[C:128, dx * CH:(dx + 1) * CH] = w1[:, :, 1, dx].T
        w1t[0:C, (3 + dx) * CH:(4 + dx) * CH] = w1[:, :, 2, dx].T
        w23t[0:C, dx * 128: dx * 128 + CH] = w2[:, :, 0, dx].T
        w23t[C:128, dx * 128: dx * 128 + CH] = w2[:, :, 1, dx].T
        w23t[0:C, dx * 128 + 64: dx * 128 + 128] = w3[:, :, 0, dx].T
        w23t[C:128, dx * 128 + 64: dx * 128 + 128] = w3[:, :, 1, dx].T
        w23t[0:C, (3 + dx) * 128: (3 + dx) * 128 + CH] = w2[:, :, 2, dx].T
        w23t[0:C, (3 + dx) * 128 + 64: (3 + dx) * 128 + 128] = w3[:, :, 2, dx].T
    wc1 = np.zeros((C, 6 * CH), np.float32)
    wc23 = np.zeros((C, 6 * 128), np.float32)
    for dy in range(3):
        for side, dx in ((0, 0), (1, 2)):
            i = 2 * dy + side
            wc1[:, i * CH:(i + 1) * CH] = -w1[:, :, dy, dx].T
            wc23[:, i * 128: i * 128 + CH] = -w2[:, :, dy, dx].T
            wc23[:, i * 128 + 64: i * 128 + 128] = -w3[:, :, dy, dx].T

    xcr = _r32r(xc)
    xbr = _r32r(xb)

    bund64 = np.zeros((C, B64_LEN), np.float32)
    bund64[:, B64_WC1:B64_WC1 + 6 * CH] = _r32r(wc1)
    bund64[:, B64_WC23:B64_WC23 + 6 * 128] = _r32r(wc23)
    bund64[:, B64_ECL:B64_ECL + 66] = xcr[0:C, (np.arange(66)) * 64]
    bund64[:, B64_ECR:B64_ECR + 67] = xcr[0:C, (np.arange(67)) * 64 + 1]
    bund64[:, B64_EBL:B64_EBL + 34] = xbr[0:C, (np.arange(34)) * 64]
    bund64[:, B64_EBR:B64_EBR + 35] = xbr[0:C, (np.arange(35)) * 64 + 1]

    bund128 = np.zeros((128, B128_LEN), np.float32)
    bund128[:, B128_W1:B128_W1 + 6 * CH] = _r32r(w1t)
    bund128[:, B128_W23:B128_W23 + 6 * 128] = _r32r(w23t)
    bund128[:, B128_B23] = np.concatenate([b2, np.zeros(32, np.float32), b3])
    bund128[0:CH, B128_B1] = b1
    bund128[0:C, B128_BV] = b3

    resid = np.ascontiguousarray(A1_C[b][:, h0:h0 + 32, :].reshape(C, M))
    return {
        "xc": xcr,
        "xb": xbr,
        "bund64": bund64,
        "bund128": bund128,
        "resid": resid,
    }


def _run(inputs, trace=False, dbg=False):
    key = ("nc", dbg)
    if key not in _cache:
        _cache[key] = _build(dbg)
    nc = _cache[key]
    in_maps = [_prep_core_inputs(inputs, i) for i in range(NCORES)]
    res = bass_utils.run_bass_kernel_spmd(
        nc, in_maps, core_ids=list(range(NCORES)), trace=trace)
    out = np.empty((B, C, H, W), np.float32)
    for i in range(NCORES):
        b, half = i // 2, i % 2
        out[b, :, half * 32:half * 32 + 32, :] = res.results[i]["out"].reshape(C, 32, W)
    return out, res


def kernel(**inputs):
    out, _ = _run(inputs, trace=False)
    return out


# revision 30
# speedup vs baseline: 1.2268x; 1.0247x over previous
"""Trainium2 Bass kernel for nn_BSAM_60129542251.

Conv-QKV self-attention block (B=4, C=64, H=W=64):
  Q = conv3x3(A1_B, w1)  -> [b, 32, 4096]
  K = conv3x3(A1_C, w2)  -> [b, 32, 4096]
  V = conv3x3(A1_C, w3)  -> [b, 64, 4096]
  E = softmax(Q^T K) V^T -> [b, 4096, 64];  out = E^T + A1_C

Sharding: 8 cores; core i handles sample b=i//2, row-half i%2 (2048 query
rows). K/V convs are duplicated within a sample pair; Q conv runs on the
core's half only. Attention is fully fused on-chip (no S matrix in HBM).

v3 structure (engine-balanced, conv/attention interleaved, software
pipelined):
  - Conv = 9 shifted matmuls done as 6 (dy-pairs stacked in contraction;
    rows 64:128 of the flat input hold a 64-shifted copy), with compact
    edge-correction matmuls cancelling the w=0/w=63 row-wrap reads.
  - Conv epilogues (PSUM->SBUF + bias) run on Pool (K, Q as fp32 bits into
    f32r tiles) and DVE (V -> bf16), keeping ACT free for exp.
  - V tiles transposed via one batched XBAR DMA transpose per conv tile
    ([64,512] -> [128, 4, 64] 3D out) straight into the [128, 65]-per-chunk
    V' stationary layout (col 64 = ones for the softmax denominator row).
  - Attention per (mt, pair-of-2-chunks): 2 QK matmuls (f32r, k chunk
    [32,128] stationary) into a [128,1024] PSUM tile, one exp -> bf16,
    2 PV matmuls (bf16) accumulating E'^T in PSUM ([65, 512], row 64 =
    denominators). exp ops are split between ACT (exact, scale=1/EXPA)
    and DVE/Pool (2^x int16-bitcast approx; EXPA folded into w1).
  - mt0's attention interleaves with the KV conv, QK one conv tile behind,
    PV two behind (hides the K-epilogue and V-transpose latency). mt1..3
    run after with a QK/PV two-stage pipeline. PSUM: 2 conv + 4 st + 2 et.
  - Normalize tail split in column halves across DVE (recip, mul) and
    Pool (broadcast, add) to shrink the exposed critical path at the end.
  - Inputs ride 2 bundled small-weight DMAs + per-half chunked xb/xc
    (sync HWDGE queue for rows 0:64, gpsimd SWDGE for rows 64:128).
"""

import numpy as np

import concourse.bacc as bacc
import concourse.mybir as mybir
import concourse.tile as tile
from concourse import bass_utils
from concourse.masks import make_identity

F32 = mybir.dt.float32
F32R = mybir.dt.float32r
BF16 = mybir.dt.bfloat16
I16 = mybir.dt.int16
AF = mybir.ActivationFunctionType

B, C, CH, H, W = 4, 64, 32, 64, 64
N = H * W                     # 4096 keys
M = N // 2                    # 2048 query rows per core
NCORES = 8
XC_LEN = 4352                 # padded flat A1_C: 66*64+2 = 4226, padded up
XB_LEN = 2304                 # padded flat A1_B half: 34*64+2 = 2178, padded up
NKC = N // 128                # 32 key chunks
NPAIR = NKC // 2              # 16 chunk pairs (one [128,1024] st tile each)

# bund64 column offsets: wc1 | wc23 | ecl | ecr | ebl | ebr
B64_WC1, B64_WC23, B64_ECL, B64_ECR, B64_EBL, B64_EBR, B64_LEN = (
    0, 192, 960, 1026, 1093, 1127, 1162)
# bund128 column offsets: w1t | w23t | b23 | b1 | bv (V bias at rows 0:64)
B128_W1, B128_W23, B128_B23, B128_B1, B128_BV, B128_LEN = 0, 192, 960, 961, 962, 963

# 2^x trick constants: S_psum = EXPA * s (EXPA folded into w1 on host);
# p = exp(s) ~ bitcast_bf16(int16(S_psum + EXPB)).
EXPA = 128.0 / float(np.log(2.0))          # 184.664965...
EXPB_ROUND = 16256.0 - 5.5053              # round-to-nearest int16 convert
EXPB_TRUNC = EXPB_ROUND + 0.5              # truncate-toward-zero convert


def _exp_engine(mt, p):
    """exp engine for (mt, pair): ACT exact, or DVE 2^x approx."""
    if p % 3 == 2:
        return "dve"
    return "act"


_cache = {}


def _r32r(x):
    """Round fp32 -> float32r (zero low 12 mantissa bits, round to nearest)."""
    x = np.ascontiguousarray(x, np.float32)
    b = x.view(np.uint32).astype(np.uint64)
    out = (((b + np.uint64(1 << 11)) & np.uint64(0xFFFFF000)).astype(np.uint32)).view(np.float32)
    return np.ascontiguousarray(out)


def _build(dbg=False, expb=EXPB_TRUNC):
    nc = bacc.Bacc("TRN2", target_bir_lowering=False, debug=False)

    xc = nc.dram_tensor("xc", [128, XC_LEN], F32R, kind="ExternalInput")
    xb = nc.dram_tensor("xb", [128, XB_LEN], F32R, kind="ExternalInput")
    bund64 = nc.dram_tensor("bund64", [C, B64_LEN], F32R, kind="ExternalInput")
    bund128 = nc.dram_tensor("bund128", [128, B128_LEN], F32R, kind="ExternalInput")
    resid = nc.dram_tensor("resid", [C, M], F32, kind="ExternalInput")
    out_d = nc.dram_tensor("out", [C, M], F32, kind="ExternalOutput")
    if dbg:
        k_d = nc.dram_tensor("k_dbg", [CH, N], F32, kind="ExternalOutput")
        q_d = nc.dram_tensor("q_dbg", [CH, M], F32, kind="ExternalOutput")
        v_d = nc.dram_tensor("v_dbg", [128, NKC * 65], F32, kind="ExternalOutput")

    with tile.TileContext(nc) as tc:
        with (
            tc.tile_pool(name="big", bufs=1) as big,
            tc.tile_pool(name="work", bufs=2) as work,
            tc.tile_pool(name="expool", bufs=6) as expool,
        ):
            xc_sb = big.tile([128, XC_LEN], F32R, tag="xc")
            xb_sb = big.tile([128, XB_LEN], F32R, tag="xb")
            b64_sb = big.tile([C, B64_LEN], F32R, tag="b64")
            b128_sb = big.tile([128, B128_LEN], F32R, tag="b128")
            res_sb = big.tile([C, M], F32, tag="res")
            corrS = big.tile([C, 320], F32, tag="corrS")
            k_sb = big.tile([CH, N], F32R, tag="k")
            qt_sb = big.tile([CH, M], F32R, tag="qt")
            v_sb = big.tile([128, NKC * 65], BF16, tag="v")

            wc1_sb = b64_sb[:, B64_WC1:B64_WC1 + 6 * CH]
            wc23_sb = b64_sb[:, B64_WC23:B64_WC23 + 6 * 128]
            ecl_sb = b64_sb[:, B64_ECL:B64_ECL + 66]
            ecr_sb = b64_sb[:, B64_ECR:B64_ECR + 67]
            ebl_sb = b64_sb[:, B64_EBL:B64_EBL + 34]
            ebr_sb = b64_sb[:, B64_EBR:B64_EBR + 35]
            w1_sb = b128_sb[:, B128_W1:B128_W1 + 6 * CH]
            w23_sb = b128_sb[:, B128_W23:B128_W23 + 6 * 128]
            b23_sb = b128_sb[:, B128_B23:B128_B23 + 1].bitcast(F32)
            b1_sb = b128_sb[0:CH, B128_B1:B128_B1 + 1].bitcast(F32)
            bv_sb = b128_sb[0:C, B128_BV:B128_BV + 1].bitcast(F32)

            # ---- input DMA head ----
            # sync/HWDGE: bundles + rows 0:64 of xb/xc (chunked);
            # gpsimd/SWDGE (Pool desc-gen): rows 64:128 + resid.
            # order: corr bundle, weights, xb head (Q0), then xc chunks
            # interleaved ahead of the remaining xb (Q1..3 run last)
            nc.sync.dma_start(out=b64_sb[:], in_=bund64.ap())
            nc.sync.dma_start(out=b128_sb[:], in_=bund128.ap())
            def _xb(lo, hi):
                nc.gpsimd.dma_start(out=xb_sb[C:128, lo:hi], in_=xb.ap()[C:128, lo:hi])
                nc.sync.dma_start(out=xb_sb[0:C, lo:hi], in_=xb.ap()[0:C, lo:hi])
            def _xc(lo, hi):
                nc.gpsimd.dma_start(out=xc_sb[C:128, lo:hi], in_=xc.ap()[C:128, lo:hi])
                nc.sync.dma_start(out=xc_sb[0:C, lo:hi], in_=xc.ap()[0:C, lo:hi])
            _xb(0, 704)
            _xc(0, 1152)
            _xc(1152, 2304)
            _xb(704, 1408)
            _xc(2304, 3456)
            _xb(1408, XB_LEN)
            _xc(3456, XC_LEN)

            # ones columns of V' (col 64 of each chunk) + exp table warm
            ones_bf = big.tile([128, NKC], BF16, tag="ones")
            nc.vector.memset(ones_bf[:], 1.0)
            v3 = v_sb[:].rearrange("p (a b) -> p a b", b=65)
            nc.vector.tensor_copy(v3[:, :, 64], ones_bf[:])
            warm = big.tile([128, 1], F32, tag="warm")
            nc.scalar.activation(warm[:], b23_sb[:], AF.Exp)
            identb = big.tile([C, C], BF16, tag="identb")
            make_identity(nc, identb[:])

            # ---- edge-correction matmuls (own PSUM bank, freed early) ----
            # layout (SBUF partition-aligned with consumers):
            #   Q-L [0:32, 0:32]   Q-R [0:32, 32:64]
            #   K-L [0:32, 64:128] K-R [0:32, 128:192]
            #   V-L [0:64, 192:256] V-R [0:64, 256:320]
            with tc.tile_pool(name="vtp", bufs=1, space="PSUM") as vtp:
                # PE pstate warmup: dummy transposes (no input deps) keep the
                # tensor engine continuously busy through the DMA head so the
                # 3us ramp to full clock completes before the real conv.
                zwarm = big.tile([C, C], BF16, tag="zwarm")
                nc.vector.memset(zwarm[:], 0.0)
                pwarm = vtp.tile([C, C], BF16, tag="pwarm")
                for _ in range(44):
                    nc.tensor.transpose(pwarm[:], zwarm[:], zwarm[:])
                corrT = vtp.tile([C, 320], F32, tag="corrT")
                mms = []
                for dy in range(3):
                    # V first: the group's start zero-region must span the
                    # full partition range (0:64) of the tile.
                    mms.append((corrT[0:C, 192:256],
                                wc23_sb[:, (2 * dy) * 128 + 64:(2 * dy) * 128 + 128],
                                ecl_sb[:, dy:dy + 64]))
                    mms.append((corrT[0:C, 256:320],
                                wc23_sb[:, (2 * dy + 1) * 128 + 64:(2 * dy + 1) * 128 + 128],
                                ecr_sb[:, dy + 1:dy + 65]))
                    mms.append((corrT[0:CH, 0:32],
                                wc1_sb[:, (2 * dy) * CH:(2 * dy + 1) * CH],
                                ebl_sb[:, dy:dy + 32]))
                    mms.append((corrT[0:CH, 32:64],
                                wc1_sb[:, (2 * dy + 1) * CH:(2 * dy + 2) * CH],
                                ebr_sb[:, dy + 1:dy + 33]))
                    mms.append((corrT[0:CH, 64:128],
                                wc23_sb[:, (2 * dy) * 128:(2 * dy) * 128 + CH],
                                ecl_sb[:, dy:dy + 64]))
                    mms.append((corrT[0:CH, 128:192],
                                wc23_sb[:, (2 * dy + 1) * 128:(2 * dy + 1) * 128 + CH],
                                ecr_sb[:, dy + 1:dy + 65]))
                # first and last matmul must span the full 0:64 partition
                # range: start's zero region and stop's group close both
                # cover only the instruction's own partitions.
                mms = [mms[0]] + mms[2:] + [mms[1]]
                for i_mm, (o, l, r) in enumerate(mms):
                    nc.tensor.matmul(o, l.bitcast(F32), r.bitcast(F32),
                                     start=(i_mm == 0), stop=(i_mm == len(mms) - 1))
                nc.vector.tensor_copy(corrS[0:CH, 0:192], corrT[0:CH, 0:192])
                nc.vector.tensor_copy(corrS[0:C, 192:320], corrT[0:C, 192:320])
            corrQ = corrS[0:CH, 0:64]

            def conv_tile(pk, x_sb, w_sb, j, co):
                for dx in range(3):
                    base = j * 512 + dx
                    nc.tensor.matmul(
                        pk[:], w_sb[:, dx * co:(dx + 1) * co],
                        x_sb[:, base:base + 512],
                        start=(dx == 0), stop=False,
                    )
                for dx in range(3):
                    base = j * 512 + 2 * 64 + dx
                    nc.tensor.matmul(
                        pk[:], w_sb[0:C, (3 + dx) * co:(4 + dx) * co],
                        x_sb[0:C, base:base + 512],
                        start=False, stop=(dx == 2),
                    )

            # ---- conv + attention ----

            with tc.tile_pool(name="eps", bufs=2, space="PSUM") as eps:
                ets = {}

                def attn_qk(mt, p, pool):
                    st = pool.tile([128, 1024], F32, tag="st")
                    for i in range(2):
                        kk = 2 * p + i
                        nc.tensor.matmul(
                            st[:, i * 512:(i + 1) * 512],
                            k_sb[:, kk * 128:(kk + 1) * 128],
                            qt_sb[:, mt * 512:(mt + 1) * 512],
                            start=True, stop=True,
                        )
                    ex = expool.tile([128, 1024], BF16, tag="ex")
                    if _exp_engine(mt, p) == "act":
                        nc.scalar.activation(ex[:], st[:], AF.Exp, scale=1.0 / EXPA)
                    else:
                        nc.vector.tensor_scalar_add(ex[:].bitcast(I16), st[:], expb)
                    return ex

                def attn_pv(mt, p, ex):
                    for i in range(2):
                        kk = 2 * p + i
                        nc.tensor.matmul(
                            ets[mt][0:C + 1, :],
                            v_sb[:, kk * 65:kk * 65 + 65],
                            ex[:, i * 512:(i + 1) * 512],
                            start=(kk == 0), stop=(kk == NKC - 1),
                        )

                def attn_tail(mt):
                    et = ets[mt]
                    sls = [slice(h * 256, (h + 1) * 256) for h in range(2)]
                    osls = [slice(mt * 512 + h * 256, mt * 512 + (h + 1) * 256)
                            for h in range(2)]
                    ots = []
                    bcs = []
                    for h in range(2):
                        recip = work.tile([1, 256], F32, tag="recip")
                        nc.vector.reciprocal(recip[:], et[C:C + 1, sls[h]])
                        bc = work.tile([C, 256], F32, tag="bc")
                        nc.gpsimd.partition_broadcast(bc[:], recip[:])
                        bcs.append(bc)
                    for h in range(2):
                        ot = work.tile([C, 256], F32, tag="ot")
                        nc.vector.tensor_mul(ot[:], et[0:C, sls[h]], bcs[h][:])
                        ots.append(ot)
                    # h0's residual add on Pool, h1's on DVE: the two finish
                    # in parallel right behind the muls
                    nc.gpsimd.tensor_add(ots[0][:], ots[0][:], res_sb[:, osls[0]])
                    nc.sync.dma_start(out=out_d.ap()[:, osls[0]], in_=ots[0][:])
                    nc.vector.tensor_add(ots[1][:], ots[1][:], res_sb[:, osls[1]])
                    nc.sync.dma_start(out=out_d.ap()[:, osls[1]], in_=ots[1][:])

                # ---- conv phase (attention PSUM not yet open) ----
                with (
                    tc.tile_pool(name="cps", bufs=3, space="PSUM") as cps,
                    tc.tile_pool(name="vtp2", bufs=2, space="PSUM") as vtp2,
                ):
                    def q_conv_tile(j):
                        pq0 = cps.tile([128, 512], F32, tag="cv")
                        pq = pq0[0:CH, :]
                        conv_tile(pq, xb_sb, w1_sb, j, CH)
                        qsl = qt_sb[:, j * 512:(j + 1) * 512]
                        nc.vector.tensor_scalar_add(qsl, pq[:], b1_sb)
                        q3 = qsl.rearrange("p (h w) -> p h w", w=64)
                        nc.vector.tensor_add(q3[:, :, 0], q3[:, :, 0], corrQ[:, j * 8:(j + 1) * 8])
                        nc.vector.tensor_add(q3[:, :, 63], q3[:, :, 63], corrQ[:, 32 + j * 8:32 + (j + 1) * 8])

                    def kv_conv_tile(j):
                        pk = cps.tile([128, 512], F32, tag="cv")
                        conv_tile(pk, xc_sb, w23_sb, j, 128)
                        ksl = k_sb[:, j * 512:(j + 1) * 512]
                        nc.vector.tensor_scalar_add(ksl, pk[0:CH, :], b23_sb[0:CH, :])
                        k3 = ksl.rearrange("p (h w) -> p h w", w=64)
                        nc.gpsimd.tensor_add(k3[:, :, 0], k3[:, :, 0], corrS[0:CH, 64 + j * 8:64 + (j + 1) * 8])
                        nc.gpsimd.tensor_add(k3[:, :, 63], k3[:, :, 63], corrS[0:CH, 128 + j * 8:128 + (j + 1) * 8])
                        vtmp = work.tile([C, 512], BF16, tag="vtmp")
                        nc.scalar.activation(vtmp[:], pk[64:128, :], AF.Identity, bias=bv_sb)
                        v3t = vtmp[:].rearrange("p (h w) -> p h w", w=64)
                        nc.vector.tensor_add(v3t[:, :, 0], v3t[:, :, 0], corrS[0:C, 192 + j * 8:192 + (j + 1) * 8])
                        nc.vector.tensor_add(v3t[:, :, 63], v3t[:, :, 63], corrS[0:C, 256 + j * 8:256 + (j + 1) * 8])
                        return vtmp

                    def v_transpose(j, vtmp):
                        vt = vtp2.tile([128, 256], BF16, tag="vt")
                        for c4 in range(4):
                            nc.tensor.transpose(
                                vt[:, c4 * 64:(c4 + 1) * 64],
                                vtmp[:, c4 * 128:(c4 + 1) * 128], identb[:])
                        vslc = v_sb[:, j * 260:(j + 1) * 260]
                        vdst = vslc.rearrange("p (c f) -> p c f", f=65)[:, :, 0:64]
                        nc.scalar.activation(vdst, vt[:].rearrange("p (c f) -> p c f", f=64),
                                             AF.Identity)

                    q_conv_tile(0)
                    vtmps = {}
                    for j in range(8):
                        vtmps[j] = kv_conv_tile(j)
                        if j >= 2:
                            v_transpose(j - 2, vtmps.pop(j - 2))
                    v_transpose(6, vtmps.pop(6))
                    q_conv_tile(1)
                    v_transpose(7, vtmps.pop(7))
                    for j in (2, 3):
                        q_conv_tile(j)
                    nc.gpsimd.dma_start(out=res_sb[:], in_=resid.ap())

                if dbg:
                    nc.sync.dma_start(out=k_d.ap(), in_=k_sb[:].bitcast(F32))
                    nc.sync.dma_start(out=q_d.ap(), in_=qt_sb[:].bitcast(F32))
                    vdbg = big.tile([128, NKC * 65], F32, tag="vdbg")
                    nc.vector.tensor_copy(vdbg[:], v_sb[:])
                    nc.sync.dma_start(out=v_d.ap(), in_=vdbg[:])

                # ---- attention phase: all 4 mts, three-stage pipeline ----
                with tc.tile_pool(name="aps2", bufs=3, space="PSUM") as aps2:
                    for mt in range(4):
                        etm = eps.tile([128, 512], F32, tag="et")
                        ets[mt] = etm
                        pend = []
                        for p in range(NPAIR):
                            pend.append((p, attn_qk(mt, p, aps2)))
                            if p == 1 and mt > 0:
                                attn_tail(mt - 1)
                            if len(pend) > 2:
                                pp, pex = pend.pop(0)
                                attn_pv(mt, pp, pex)
                        for pp, pex in pend:
                            attn_pv(mt, pp, pex)
                    attn_tail(3)

    nc.compile()
    return nc


def _prep_core_inputs(inputs, core):
    A1_B = np.asarray(inputs["A1_B"], np.float32)
    A1_C = np.asarray(inputs["A1_C"], np.float32)
    w1 = np.asarray(inputs["w1"], np.float32) * EXPA
    b1 = np.asarray(inputs["b1"], np.float32) * EXPA
    w2 = np.asarray(inputs["w2"], np.float32)
    b2 = np.asarray(inputs["b2"], np.float32)
    w3 = np.asarray(inputs["w3"], np.float32)
    b3 = np.asarray(inputs["b3"], np.float32)
    b = core // 2
    half = core % 2
    h0 = half * 32

    xc = np.zeros((128, XC_LEN), np.float32)
    flat_c = np.zeros((C, H + 2, W), np.float32)
    flat_c[:, 1:H + 1, :] = A1_C[b]
    xc[0:C, 1:1 + (H + 2) * W] = flat_c.reshape(C, -1)
    xc[C:128, 0:XC_LEN - 64] = xc[0:C, 64:XC_LEN]

    xb = np.zeros((128, XB_LEN), np.float32)
    flat_b = np.zeros((C, 34, W), np.float32)
    glo = h0 - 1
    src_lo = max(glo, 0)
    src_hi = min(h0 + 33, H)
    flat_b[:, src_lo - glo: src_hi - glo, :] = A1_B[b][:, src_lo:src_hi, :]
    xb[0:C, 1:1 + 34 * W] = flat_b.reshape(C, -1)
    xb[C:128, 0:XB_LEN - 64] = xb[0:C, 64:XB_LEN]

    # Tap layout: cols t*co.. with t=0..2 the (dy=0,dy=1) stacked pairs
    # (contraction rows 0:64 = dy0, 64:128 = dy1), t=3..5 the dy=2 singles.
    w1t = np.zeros((128, 6 * CH), np.float32)
    w23t = np.zeros((128, 6 * 128), np.float32)
    for dx in range(3):
        w1t[0:C, dx * CH:(dx + 1) * CH] = w1[:, :, 0, dx].T
        w1t[C:128, dx * CH:(dx + 1) * CH] = w1[:, :, 1, dx].T
        w1t[0:C, (3 + dx) * CH:(4 + dx) * CH] = w1[:, :, 2, dx].T
        w23t[0:C, dx * 128: dx * 128 + CH] = w2[:, :, 0, dx].T
        w23t[C:128, dx * 128: dx * 128 + CH] = w2[:, :, 1, dx].T
        w23t[0:C, dx * 128 + 64: dx * 128 + 128] = w3[:, :, 0, dx].T
        w23t[C:128, dx * 128 + 64: dx * 128 + 128] = w3[:, :, 1, dx].T
        w23t[0:C, (3 + dx) * 128: (3 + dx) * 128 + CH] = w2[:, :, 2, dx].T
        w23t[0:C, (3 + dx) * 128 + 64: (3 + dx) * 128 + 128] = w3[:, :, 2, dx].T
    wc1 = np.zeros((C, 6 * CH), np.float32)
    wc23 = np.zeros((C, 6 * 128), np.float32)
    for dy in range(3):
        for side, dx in ((0, 0), (1, 2)):
            i = 2 * dy + side
            wc1[:, i * CH:(i + 1) * CH] = -w1[:, :, dy, dx].T
            wc23[:, i * 128: i * 128 + CH] = -w2[:, :, dy, dx].T
            wc23[:, i * 128 + 64: i * 128 + 128] = -w3[:, :, dy, dx].T

    xcr = _r32r(xc)
    xbr = _r32r(xb)

    bund64 = np.zeros((C, B64_LEN), np.float32)
    bund64[:, B64_WC1:B64_WC1 + 6 * CH] = _r32r(wc1)
    bund64[:, B64_WC23:B64_WC23 + 6 * 128] = _r32r(wc23)
    bund64[:, B64_ECL:B64_ECL + 66] = xcr[0:C, (np.arange(66)) * 64]
    bund64[:, B64_ECR:B64_ECR + 67] = xcr[0:C, (np.arange(67)) * 64 + 1]
    bund64[:, B64_EBL:B64_EBL + 34] = xbr[0:C, (np.arange(34)) * 64]
    bund64[:, B64_EBR:B64_EBR + 35] = xbr[0:C, (np.arange(35)) * 64 + 1]

    bund128 = np.zeros((128, B128_LEN), np.float32)
    bund128[:, B128_W1:B128_W1 + 6 * CH] = _r32r(w1t)
    bund128[:, B128_W23:B128_W23 + 6 * 128] = _r32r(w23t)
    bund128[:, B128_B23] = np.concatenate([b2, np.zeros(32, np.float32), b3])
    bund128[0:CH, B128_B1] = b1
    bund128[0:C, B128_BV] = b3

    resid = np.ascontiguousarray(A1_C[b][:, h0:h0 + 32, :].reshape(C, M))
    return {
        "xc": xcr,
        "xb": xbr,
        "bund64": bund64,
        "bund128": bund128,
        "resid": resid,
    }


def _run(inputs, trace=False, dbg=False):
    key = ("nc", dbg)
    if key not in _cache:
        _cache[key] = _build(dbg)
    nc = _cache[key]
    in_maps = [_prep_core_inputs(inputs, i) for i in range(NCORES)]
    res = bass_utils.run_bass_kernel_spmd(
        nc, in_maps, core_ids=list(range(NCORES)), trace=trace)
    out = np.empty((B, C, H, W), np.float32)
    for i in range(NCORES):
        b, half = i // 2, i % 2
        out[b, :, half * 32:half * 32 + 32, :] = res.results[i]["out"].reshape(C, 32, W)
    return out, res


def kernel(**inputs):
    out, _ = _run(inputs, trace=False)
    return out
